# revision 1
# baseline (speedup 1.0000x reference)
"""MLP-Mixer forward on 8 Trainium2 NeuronCores, data-parallel over batch.

Strategy
--------
Pure data parallel: 64 samples -> 8 per core; all parameters replicated.
Per core, the 8 residual states h[s] (427x427 fp32) stay resident in SBUF
across all 8 mixer blocks; block weights stream from HBM (bf16, halved
traffic) into double-buffered SBUF slots so each phase's weight DMA fully
overlaps the previous phase's compute.

All matmuls run in bf16 (1 cyc/row on PE, same rate as f32r but with no
even-moving-dim constraint -> N=427) with fp32 PSUM accumulation; the
fp32 residual stream h keeps accumulation exact.  Verified numerically:
quantizing every matmul operand to bf16 gives ~4.5e-3 max rel err vs the
2e-2 budget.

Layouts (per sample, all "row tiles" = 4 chunks of 128|43 partitions):
  h    [128, 4, 428] fp32   rows H, cols W         (residual stream)
  hn   [128, 4, 428] bf16   LN1(h), K=H for MM1
  gy   [128, 428]    bf16   gelu(MM1-out chunk), K for MM2 (4 rotating)
  token: yT = w1.T @ hn  (M=t);  zT = w2.T @ gy (M=d);  h += zT + b2
         (out lands in h layout, no transposes)
  channel: y2n = LN2(h) bf16; y2nT = PE-transpose(y2n) (bf16 identity,
           1 cyc/row); uT = c1.T @ y2nT; gu = gelu(uT);
           MM2 SWAPPED: v = gu.T @ c2 with gu as the STATIONARY operand
           and the c2 rows as the MOVING operand -> out [H-tile, W] is
           already in h layout; h += v (+cb2 broadcast) with no evict
           transpose at all.
  final: f1 = LNf(h) bf16; f1T = transpose(f1); h2 = f1T.T @ lw (+lb);
         out = |LNf(h2)| * (1 - eye)  via Abs(x*rstd - m*rstd), mask mul.

LayerNorm: single-pass mean+var via DVE bn_stats/bn_aggr, rstd via
Newton sqrt on DVE (reciprocal-based, 4 iters) -- keeps ScalarE on the
gelu table set the whole run (no 2.7us ACT table switches).  LN affine
(g,b), matmul biases and lb are skipped when the provided values are
exactly ones/zeros (true for this model's inputs); otherwise applied via
host-broadcast tiles.

Scheduling: one flat software-pipelined driver over all 17 phases.
LN/transpose prep runs two samples ahead of its matmul chain, the next
phase's first preps are hoisted into the current phase's tail, the
interleaved second matmul lags MM1 by two m-steps (so the previous
sample's residual-add can release the PSUM accumulators in time), and
the final stage is zipped into the last channel phase so its DVE-heavy
LN/abs/mask path and the output DMAs hide under the remaining chains.
TimelineSim: 3.079 ms/core (PE 97.4% busy; pure-matmul floor 2.92 ms).
"""

import os
from contextlib import ExitStack

import numpy as np
import ml_dtypes

import concourse.bacc as bacc
import concourse.tile as tile
import concourse.mybir as mybir
from concourse.bass import ts
from concourse.bass_utils import run_bass_kernel_spmd
from concourse.masks import make_identity

B, C, DIM, DEPTH, TOK = 64, 3, 427, 8, 2048
NCORES = 8
SPC = B // NCORES           # samples per core
P = 128
WP = DIM + 1                # 428, tile stride (matmuls use :DIM)
NT = TOK // P               # 16
RT = [(0, 128), (128, 128), (256, 128), (384, 43)]  # DIM row/k tiles
NR = len(RT)
EPS = 1e-5
F32 = mybir.dt.float32
BF16 = mybir.dt.bfloat16
AF = mybir.ActivationFunctionType
ALU = mybir.AluOpType
AX = mybir.AxisListType

_BUILD_CACHE = {}


def _build(ln1_triv, ln2_triv, lnf_triv, lb_triv, b1_triv, b2_triv,
           reps=1):
    nc = bacc.Bacc("TRN2", target_bir_lowering=False, debug=False,
                   num_devices=NCORES)

    x_d = nc.dram_tensor("x", [SPC, C, DIM, DIM], F32, kind="ExternalInput").ap()
    rw1_d = nc.dram_tensor("rw1", [DEPTH, DIM, TOK], BF16, kind="ExternalInput").ap()
    rw2_d = nc.dram_tensor("rw2", [DEPTH, TOK, DIM], BF16, kind="ExternalInput").ap()
    cw1_d = nc.dram_tensor("cw1", [DEPTH, DIM, TOK], BF16, kind="ExternalInput").ap()
    cw2_d = nc.dram_tensor("cw2", [DEPTH, TOK, DIM], BF16, kind="ExternalInput").ap()
    lw_d = nc.dram_tensor("lw", [DIM, DIM], BF16, kind="ExternalInput").ap()
    dmask_d = nc.dram_tensor("dmask", [NR, P, DIM], F32, kind="ExternalInput").ap()
    rb1_d = rb2_d = cb1_d = cb2b_d = None
    if not b1_triv:
        rb1_d = nc.dram_tensor("rb1", [DEPTH, TOK], F32, kind="ExternalInput").ap()
        cb1_d = nc.dram_tensor("cb1", [DEPTH, TOK], F32, kind="ExternalInput").ap()
    if not b2_triv:
        rb2_d = nc.dram_tensor("rb2", [DEPTH, DIM], F32, kind="ExternalInput").ap()
        # channel second-bias, broadcast along partitions (free-axis bias)
        cb2b_d = nc.dram_tensor("cb2b", [DEPTH, P, DIM], F32, kind="ExternalInput").ap()
    # broadcast LN affine / lb params (used only when nontrivial)
    ln1g_d = ln2g_d = lnfg_d = lb_d = None
    if not ln1_triv:
        ln1g_d = nc.dram_tensor("ln1gb", [DEPTH, 2, P, DIM], F32, kind="ExternalInput").ap()
    if not ln2_triv:
        ln2g_d = nc.dram_tensor("ln2gb", [DEPTH, 2, P, DIM], F32, kind="ExternalInput").ap()
    if not lnf_triv:
        lnfg_d = nc.dram_tensor("lnfgb", [2, P, DIM], F32, kind="ExternalInput").ap()
    if not lb_triv:
        lb_d = nc.dram_tensor("lbb", [P, DIM], F32, kind="ExternalInput").ap()
    out_d = nc.dram_tensor("out", [SPC, DIM, DIM], F32, kind="ExternalOutput").ap()

    with tile.TileContext(nc) as tc, ExitStack() as ctx:
        hp = ctx.enter_context(tc.tile_pool(name="h", bufs=SPC))
        wa = ctx.enter_context(tc.tile_pool(name="wa", bufs=2))
        wb = ctx.enter_context(tc.tile_pool(name="wb", bufs=2))
        hnp = ctx.enter_context(tc.tile_pool(name="hn", bufs=4))
        y2p = ctx.enter_context(tc.tile_pool(name="y2n", bufs=6))
        ytp = ctx.enter_context(tc.tile_pool(name="y2nT", bufs=6))
        vbp = ctx.enter_context(tc.tile_pool(name="vb", bufs=1))
        otp = ctx.enter_context(tc.tile_pool(name="ot", bufs=2))
        gyp = ctx.enter_context(tc.tile_pool(name="gy", bufs=4))
        sqp = ctx.enter_context(tc.tile_pool(name="sq", bufs=1))
        xpp = ctx.enter_context(tc.tile_pool(name="xp", bufs=1))
        cst = ctx.enter_context(tc.tile_pool(name="cst", bufs=1))
        bp = ctx.enter_context(tc.tile_pool(name="bias", bufs=2))
        stp = ctx.enter_context(tc.tile_pool(name="st", bufs=2))
        gbp = ctx.enter_context(tc.tile_pool(name="gb", bufs=2))
        ps_mo = ctx.enter_context(tc.tile_pool(name="mo", bufs=2, space="PSUM"))
        ps_acc = ctx.enter_context(tc.tile_pool(name="acc", bufs=4, space="PSUM"))
        ps_tp = ctx.enter_context(tc.tile_pool(name="tp", bufs=2, space="PSUM"))

        # persistent tiles
        h_t = [hp.tile([P, NR, WP], F32, tag="h", name=f"h{i}") for i in range(SPC)]
        hn_t = [hnp.tile([P, NR, WP], BF16, tag="hn", name=f"hn{i}") for i in range(4)]
        gy_t = [gyp.tile([P, WP], BF16, tag="gy", name=f"gy{i}") for i in range(4)]
        ident = cst.tile([P, P], BF16, tag="ident")
        make_identity(nc, ident[:, :])
        dm_t = cst.tile([P, NR, DIM], F32, tag="dmask")
        dm_loaded = [False]

        def load_dmask():
            if not dm_loaded[0]:
                dm_loaded[0] = True
                for r in range(NR):
                    nc.sync.dma_start(dm_t[:, r, :], dmask_d[r])

        def ln_stats(srcs, want_nmr=False):
            """srcs: list of NR APs [rsz, DIM]. Returns (negmean, rstd[, nmr])
            as [P, NR] tiles (column r = row-tile r)."""
            st6 = stp.tile([P, NR, 6], F32, tag="st6")
            mv = stp.tile([P, NR, 2], F32, tag="mv")
            for r, src in enumerate(srcs):
                rsz = RT[r][1]
                # single-pass mean+variance on DVE (BN stats)
                nc.vector.bn_stats(st6[:rsz, r, :], src)
                nc.vector.bn_aggr(mv[:rsz, r, :], st6[:rsz, r, :])
            var = stp.tile([P, NR], F32, tag="var")
            y = stp.tile([P, NR], F32, tag="nwy")
            q = stp.tile([P, NR], F32, tag="nwq")
            rstd = stp.tile([P, NR], F32, tag="rstd")
            nc.vector.tensor_scalar(var[:, :], mv[:, :, 1], EPS, None, ALU.add)
            # Newton sqrt: y0 = 0.5*(1+v); y <- 0.5*(y + v/y)  (4 iters)
            nc.vector.tensor_scalar(y[:, :], var[:, :], 1.0, 0.5, ALU.add, ALU.mult)
            for _ in range(4):
                nc.vector.reciprocal(q[:, :], y[:, :])
                nc.vector.tensor_tensor(q[:, :], var[:, :], q[:, :], ALU.mult)
                nc.vector.tensor_tensor(y[:, :], y[:, :], q[:, :], ALU.add)
                nc.vector.tensor_scalar(y[:, :], y[:, :], 0.5, None, ALU.mult)
            nc.vector.reciprocal(rstd[:, :], y[:, :])
            if want_nmr:
                nmr = stp.tile([P, NR], F32, tag="nmr")
                nc.vector.tensor_tensor(nmr[:, :], mv[:, :, 0], rstd[:, :],
                                        ALU.mult)
                nc.vector.tensor_scalar(nmr[:, :], nmr[:, :], -1.0, None,
                                        ALU.mult)
                return mv, rstd, nmr
            return mv, rstd

        def ln_apply(srcs, dst, mv, rstd, gb_tile):
            """dst[:rsz, r, :DIM] = LN of srcs[r]; gb_tile [2, P, DIM]-style
            sbuf tile ([P, 2, DIM]) or None for trivial affine."""
            for r in range(NR):
                rsz = RT[r][1]
                nc.vector.tensor_scalar(
                    dst[:rsz, r, :DIM], srcs[r], mv[:rsz, r, 0:1],
                    rstd[:rsz, r:r + 1], ALU.subtract, ALU.mult)
                if gb_tile is not None:
                    nc.vector.tensor_tensor(
                        dst[:rsz, r, :DIM], dst[:rsz, r, :DIM],
                        gb_tile[:rsz, 0, :DIM], ALU.mult)
                    nc.vector.tensor_tensor(
                        dst[:rsz, r, :DIM], dst[:rsz, r, :DIM],
                        gb_tile[:rsz, 1, :DIM], ALU.add)

        def load_gb(dram_ap, tag):
            t = gbp.tile([P, 2, DIM], F32, tag=tag)
            nc.sync.dma_start(t[:, 0, :], dram_ap[0])
            nc.sync.dma_start(t[:, 1, :], dram_ap[1])
            return t

        def h_rows(s):
            return [h_t[s][:RT[r][1], r, :DIM] for r in range(NR)]

        def xprep(s):
            for r in range(NR):
                r0, rsz = RT[r]
                xs = []
                for c in range(C):
                    xt = xpp.tile([P, WP], F32, tag=f"xp{c}")
                    nc.sync.dma_start(xt[:rsz, :DIM], x_d[s, c, r0:r0 + rsz, :])
                    xs.append(xt)
                hr = h_t[s][:rsz, r, :DIM]
                nc.vector.tensor_tensor(hr, xs[0][:rsz, :DIM], xs[1][:rsz, :DIM], ALU.add)
                nc.vector.tensor_tensor(hr, hr, xs[2][:rsz, :DIM], ALU.add)
                nc.vector.tensor_scalar(hr, hr, 1.0 / C, None, ALU.mult)

        def load_w1like(dram_ap):
            """[DIM, TOK] -> [P, NR, TOK] bf16, chunked DMA for JIT streaming."""
            t = wa.tile([P, NR, TOK], BF16, tag="wa")
            CH_ = 512
            for m0 in range(0, TOK, CH_):
                for k in range(NR):
                    k0, ksz = RT[k]
                    nc.sync.dma_start(t[:ksz, k, m0:m0 + CH_],
                                      dram_ap[k0:k0 + ksz, m0:m0 + CH_])
            return t

        def load_w2like(dram_ap):
            """[TOK, DIM] -> [P, NT, DIM] bf16, per-k DMA."""
            t = wb.tile([P, NT, DIM], BF16, tag="wb")
            for k in range(NT):
                nc.sync.dma_start(t[:, k, :], dram_ap[ts(k, P), :])
            return t

        def load_b1like(dram_ap):
            t = bp.tile([P, NT], F32, tag="b1")
            nc.sync.dma_start(t[:, :], dram_ap.rearrange("(k p) -> p k", p=P))
            return t

        def load_b2like(dram_ap):
            t = bp.tile([P, NR], F32, tag="b2")
            nc.sync.dma_start(t[:, :3], dram_ap[:384].rearrange("(d p) -> p d", p=P))
            nc.sync.dma_start(t[:43, 3:4], dram_ap[384:, None])
            return t

        def mixer_mms(rhs_fn, w1t, w2t, b1t, swap2):
            """Emit the 2x matmul chain: for m: MM1(m); gelu(m); MM2(m-1).
            rhs_fn(k) -> AP [ksz, DIM] (K-chunk of first-matmul rhs).
            swap2: MM2 uses gy as STATIONARY and w2 rows as MOVING, so the
            out tiles land in [DIM-tile, W] layout without a transpose.
            Returns the 4 accumulator psum tiles of the second matmul."""
            accs = [ps_acc.tile([P, WP], F32, tag="acc", name=f"acc{i}") for i in range(NR)]

            def mm2(k):
                for d in range(NR):
                    d0, dsz = RT[d]
                    if swap2:
                        nc.tensor.matmul(accs[d][:dsz, :DIM],
                                         gy_t[k % 4][:, d0:d0 + dsz],
                                         w2t[:, k, :DIM],
                                         start=(k == 0), stop=(k == NT - 1))
                    else:
                        nc.tensor.matmul(accs[d][:dsz, :DIM],
                                         w2t[:, k, d0:d0 + dsz],
                                         gy_t[k % 4][:, :DIM],
                                         start=(k == 0), stop=(k == NT - 1))
            for m in range(NT):
                y_ps = ps_mo.tile([P, WP], F32, tag="mo")
                for k in range(NR):
                    ksz = RT[k][1]
                    nc.tensor.matmul(y_ps[:, :DIM], w1t[:ksz, k, ts(m, P)],
                                     rhs_fn(k), start=(k == 0), stop=(k == NR - 1))
                if b1t is None:
                    nc.scalar.activation(gy_t[m % 4][:, :DIM], y_ps[:, :DIM], AF.Gelu)
                else:
                    nc.scalar.activation(gy_t[m % 4][:, :DIM], y_ps[:, :DIM], AF.Gelu,
                                         bias=b1t[:, m:m + 1])
                if m >= 2:
                    mm2(m - 2)
            mm2(NT - 2)
            mm2(NT - 1)
            return accs

        def transpose_to(dst, src_tile):
            """PE-transpose src_tile ([P, NR, *] bf16, row-tiles over
            partitions) into dst ([P, NR, WP] bf16).  Column-tile outer so
            each dst k-chunk needs only one whole-row psum tile + ONE DVE
            copy (first-chunk consumers can start after 4 transposes)."""
            for c in range(NR):
                c0, csz = RT[c]
                tp = ps_tp.tile([P, WP], BF16, tag="tp")
                for r in range(NR):
                    r0, rsz = RT[r]
                    nc.tensor.transpose(tp[:csz, r0:r0 + rsz],
                                        src_tile[:rsz, r, c0:c0 + csz],
                                        ident[:rsz, :rsz])
                nc.vector.tensor_copy(dst[:csz, c, :DIM], tp[:csz, :DIM])

        def add_resid(s, accs, b2t, d2bt):
            """h[s] += accs (+ per-partition b2 | + free-axis broadcast d2).
            Runs on the otherwise-idle Pool engine: its queue is empty, so
            the psum accumulators free up right after the chain ends instead
            of waiting behind prep work in the in-order DVE queue."""
            for d in range(NR):
                dsz = RT[d][1]
                hr = h_t[s][:dsz, d, :DIM]
                nc.vector.tensor_tensor(hr, hr, accs[d][:dsz, :DIM], ALU.add)
                if b2t is not None:
                    nc.vector.tensor_scalar_add(hr, hr, b2t[:dsz, d:d + 1])
                if d2bt is not None:
                    nc.vector.tensor_tensor(hr, hr, d2bt[:dsz, :DIM], ALU.add)

        # ---------------- main program ----------------
        # All phases (token/channel per block, plus the final stage) are
        # driven by one software-pipelined loop: each phase step emits the
        # NEXT sample's LN/transpose prep before the current sample's
        # matmul chain, and the first prep of the NEXT phase is emitted
        # during the last sample of the current phase, so the serial
        # stats->newton->apply chain always hides under PE matmuls --
        # including across phase boundaries.
        depth = int(os.environ.get("KMIX_DEPTH", DEPTH))

        class _Ph:
            pass

        def tok_phase(blk):
            ph = _Ph()

            def load_small():
                ph.b1 = None if b1_triv else load_b1like(rb1_d[blk])
                ph.b2 = None if b2_triv else load_b2like(rb2_d[blk])
                ph.gb = None if ln1_triv else load_gb(ln1g_d[blk], "gb1")

            def load_big():
                ph.w1 = load_w1like(rw1_d[blk])
                ph.w2 = load_w2like(rw2_d[blk])

            def prep(s):
                if blk == 0:
                    xprep(s)
                mn, rstd = ln_stats(h_rows(s))
                t = hn_t[s % 4]
                ln_apply(h_rows(s), t, mn, rstd, ph.gb)
                return t

            def mm(s, handle):
                return mixer_mms(lambda k: handle[:RT[k][1], k, :DIM],
                                 ph.w1, ph.w2, ph.b1, swap2=False)

            def resid(s, accs):
                add_resid(s, accs, ph.b2, None)

            ph.load_small, ph.load_big = load_small, load_big
            ph.prep, ph.mm, ph.resid = prep, mm, resid
            ph.kind = "tok"
            return ph

        def ch_phase(blk):
            ph = _Ph()

            def load_small():
                ph.b1 = None if b1_triv else load_b1like(cb1_d[blk])
                ph.d2 = None
                if not b2_triv:
                    ph.d2 = gbp.tile([P, DIM], F32, tag="d2b")
                    nc.sync.dma_start(ph.d2[:, :], cb2b_d[blk])
                ph.gb = None if ln2_triv else load_gb(ln2g_d[blk], "gb2")

            def load_big():
                ph.w1 = load_w1like(cw1_d[blk])
                ph.w2 = load_w2like(cw2_d[blk])

            def prep(s):
                mn, rstd = ln_stats(h_rows(s))
                y2 = y2p.tile([P, NR, WP], BF16, tag="y2n")
                ln_apply(h_rows(s), y2, mn, rstd, ph.gb)
                ytt = ytp.tile([P, NR, WP], BF16, tag="y2nT")
                transpose_to(ytt, y2)
                return ytt

            def mm(s, handle):
                return mixer_mms(lambda k: handle[:RT[k][1], k, :DIM],
                                 ph.w1, ph.w2, ph.b1, swap2=True)

            def resid(s, accs):
                add_resid(s, accs, None, ph.d2)

            ph.load_small, ph.load_big = load_small, load_big
            ph.prep, ph.mm, ph.resid = prep, mm, resid
            ph.kind = "ch"
            return ph

        def fin_phase():
            ph = _Ph()

            def load_small():
                load_dmask()
                ph.gbf = None if lnf_triv else load_gb(lnfg_d, "gbf")
                ph.lb = None
                if not lb_triv:
                    ph.lb = gbp.tile([P, DIM], F32, tag="lbb")
                    nc.sync.dma_start(ph.lb[:, :], lb_d)

            def load_big():
                ph.lw = wb.tile([P, NR, WP], BF16, tag="wb")
                for k in range(NR):
                    k0, ksz = RT[k]
                    nc.sync.dma_start(ph.lw[:ksz, k, :DIM], lw_d[k0:k0 + ksz, :])

            def prep(s):
                if depth == 0:      # debug path: no mixer blocks ran
                    xprep(s)
                mn, rstd = ln_stats(h_rows(s))
                f1 = y2p.tile([P, NR, WP], BF16, tag="y2n")
                ln_apply(h_rows(s), f1, mn, rstd, ph.gbf)
                f1T = ytp.tile([P, NR, WP], BF16, tag="y2nT")
                transpose_to(f1T, f1)
                return f1T

            def mm(s, f1T):
                f2 = vbp.tile([P, NR, WP], F32, tag="vb")
                for m in range(NR):
                    m0, msz = RT[m]
                    acc3 = ps_mo.tile([P, WP], F32, tag="mo")
                    for c in range(NR):
                        csz = RT[c][1]
                        nc.tensor.matmul(acc3[:msz, :DIM],
                                         f1T[:csz, c, m0:m0 + msz],
                                         ph.lw[:csz, c, :DIM],
                                         start=(c == 0), stop=(c == NR - 1))
                    nc.scalar.activation(f2[:msz, m, :DIM], acc3[:msz, :DIM],
                                         AF.Identity, bias=0.0)
                    if ph.lb is not None:
                        nc.vector.tensor_tensor(f2[:msz, m, :DIM],
                                                f2[:msz, m, :DIM],
                                                ph.lb[:msz, :DIM], ALU.add)
                f2rows = [f2[:RT[r][1], r, :DIM] for r in range(NR)]
                mn2, rstd2, nmr2 = ln_stats(f2rows, want_nmr=True)
                ot = otp.tile([P, NR, WP], F32, tag="ot")
                for r in range(NR):
                    r0, rsz = RT[r]
                    if ph.gbf is None:
                        nc.scalar.activation(ot[:rsz, r, :DIM], f2rows[r], AF.Abs,
                                             scale=rstd2[:rsz, r:r + 1],
                                             bias=nmr2[:rsz, r:r + 1])
                    else:
                        # general path: LN -> affine -> abs
                        nc.vector.tensor_scalar(ot[:rsz, r, :DIM], f2rows[r],
                                                mn2[:rsz, r, 0:1],
                                                rstd2[:rsz, r:r + 1],
                                                ALU.subtract, ALU.mult)
                        nc.vector.tensor_tensor(ot[:rsz, r, :DIM],
                                                ot[:rsz, r, :DIM],
                                                ph.gbf[:rsz, 0, :DIM], ALU.mult)
                        nc.vector.tensor_tensor(ot[:rsz, r, :DIM],
                                                ot[:rsz, r, :DIM],
                                                ph.gbf[:rsz, 1, :DIM], ALU.add)
                        nc.scalar.activation(ot[:rsz, r, :DIM],
                                             ot[:rsz, r, :DIM], AF.Abs)
                    nc.vector.tensor_tensor(ot[:rsz, r, :DIM], ot[:rsz, r, :DIM],
                                            dm_t[:rsz, r, :DIM], ALU.mult)
                    nc.sync.dma_start(out_d[s, r0:r0 + rsz, :],
                                      ot[:rsz, r, :DIM])
                return None

            def resid(s, accs):
                pass

            ph.load_small, ph.load_big = load_small, load_big
            ph.prep, ph.mm, ph.resid = prep, mm, resid
            ph.kind = "fin"
            return ph

        phases = []
        for rep in range(reps):
            for blk in range(depth):
                phases.append(tok_phase(blk))
                phases.append(ch_phase(blk))
            phases.append(fin_phase())

        # Flat step list with depth-2 prep lookahead: prep(j+2) is emitted
        # before mm(j), so each sample's LN/transpose chain is two matmul
        # chains ahead of its consumer and never pokes out.  The final stage
        # is zipped into the last channel phase (offset 2).
        steps = []
        for pi, ph in enumerate(phases):
            if ph.kind == "fin" and pi > 0 and phases[pi - 1].kind == "ch":
                phases[pi - 1].zipf = ph
                continue
            for s in range(SPC):
                steps.append((ph, s))

        handles = {}

        def emit_prep(j):
            if j >= len(steps):
                return
            ph, s = steps[j]
            if not getattr(ph, "loaded", False):
                ph.load_small()
                handles[j] = ph.prep(s)
                ph.load_big()
                ph.loaded = True
            else:
                handles[j] = ph.prep(s)

        emit_prep(0)
        emit_prep(1)
        pend = None        # (resid_fn, s, accs) awaiting residual add
        for j, (ph, s) in enumerate(steps):
            emit_prep(j + 2)
            zipf = getattr(ph, "zipf", None)
            fh = None
            if zipf is not None and s >= 2:
                if not getattr(zipf, "loaded", False):
                    zipf.load_small()
                    zipf.load_big()
                    zipf.loaded = True
                fh = zipf.prep(s - 2)
            if pend is not None:
                pend[0](pend[1], pend[2])
                pend = None
            accs = ph.mm(s, handles.pop(j))
            if accs is not None:
                pend = (ph.resid, s, accs)
            if fh is not None:
                zipf.mm(s - 2, fh)
            if zipf is not None and s == SPC - 1:
                if pend is not None:
                    pend[0](pend[1], pend[2])
                    pend = None
                for s2 in (SPC - 2, SPC - 1):
                    fh = zipf.prep(s2)
                    zipf.mm(s2, fh)
        if pend is not None:
            pend[0](pend[1], pend[2])

    nc.compile()
    return nc


def _host_prep(inputs):
    g = {k: np.asarray(v, dtype=np.float32) for k, v in inputs.items()}
    ln1_triv = bool(np.all(g["ln1_g"] == 1.0) and np.all(g["ln1_b"] == 0.0))
    ln2_triv = bool(np.all(g["ln2_g"] == 1.0) and np.all(g["ln2_b"] == 0.0))
    lnf_triv = bool(np.all(g["lnf_g"] == 1.0) and np.all(g["lnf_b"] == 0.0))
    lb_triv = bool(np.all(g["lb"] == 0.0))
    b1_triv = bool(np.all(g["rb1"] == 0.0) and np.all(g["cb1"] == 0.0))
    b2_triv = bool(np.all(g["rb2"] == 0.0) and np.all(g["cb2"] == 0.0))
    flags = (ln1_triv, ln2_triv, lnf_triv, lb_triv, b1_triv, b2_triv)

    dmask = np.ones((NR, P, DIM), dtype=np.float32)
    for r in range(NR):
        for p in range(min(P, DIM - 128 * r)):
            dmask[r, p, 128 * r + p] = 0.0

    def bcast2(gv, bv):  # [..., DIM] -> [..., 2, P, DIM]
        gb = np.stack([gv, bv], axis=-2)[..., None, :]          # [...,2,1,DIM]
        return np.broadcast_to(gb, gb.shape[:-2] + (P, DIM)).copy()

    bf = ml_dtypes.bfloat16
    common = {
        "rw1": np.ascontiguousarray(g["rw1"].astype(bf)),
        "rw2": np.ascontiguousarray(g["rw2"].astype(bf)),
        "cw1": np.ascontiguousarray(g["cw1"].astype(bf)),
        "cw2": np.ascontiguousarray(g["cw2"].astype(bf)),
        "lw": np.ascontiguousarray(g["lw"].astype(bf)),
        "dmask": dmask,
    }
    if not b1_triv:
        common["rb1"] = g["rb1"]
        common["cb1"] = g["cb1"]
    if not b2_triv:
        common["rb2"] = g["rb2"]
        common["cb2b"] = np.ascontiguousarray(
            np.broadcast_to(g["cb2"][:, None, :], (DEPTH, P, DIM)))
    if not ln1_triv:
        common["ln1gb"] = np.ascontiguousarray(bcast2(g["ln1_g"], g["ln1_b"]))
    if not ln2_triv:
        common["ln2gb"] = np.ascontiguousarray(bcast2(g["ln2_g"], g["ln2_b"]))
    if not lnf_triv:
        common["lnfgb"] = np.ascontiguousarray(bcast2(g["lnf_g"], g["lnf_b"]))
    if not lb_triv:
        common["lbb"] = np.broadcast_to(g["lb"][None, :], (P, DIM)).copy()
    x = np.ascontiguousarray(g["x"])
    in_maps = [dict(common, x=np.ascontiguousarray(x[c * SPC:(c + 1) * SPC]))
               for c in range(NCORES)]
    return flags, in_maps


def _get_nc(flags, reps=1):
    key = flags + (reps,)
    if key not in _BUILD_CACHE:
        _BUILD_CACHE[key] = _build(*flags, reps=reps)
    return _BUILD_CACHE[key]


def kernel(**inputs):
    flags, in_maps = _host_prep(inputs)
    nc = _get_nc(flags)
    res = run_bass_kernel_spmd(nc, in_maps, list(range(NCORES)))
    return np.concatenate([res.results[c]["out"] for c in range(NCORES)], axis=0)


def measure_hw_time(inputs, r_hi=4, iters=5):
    """Wall-clock repetition-slope timing on the real device.

    Builds the program with 1 and r_hi forward repetitions; the slope
    (t[r_hi] - t[1]) / (r_hi - 1) cancels the per-call host/RPC/transfer
    overhead, which is identical for both builds.
    """
    import time as _time
    flags, in_maps = _host_prep(inputs)
    cores = list(range(NCORES))
    t = {}
    for r in (1, r_hi):
        nc = _get_nc(flags, reps=r)
        best = float("inf")
        run_bass_kernel_spmd(nc, in_maps, cores)  # warm (jit + neff cache)
        for _ in range(iters):
            t0 = _time.perf_counter()
            run_bass_kernel_spmd(nc, in_maps, cores)
            best = min(best, _time.perf_counter() - t0)
        t[r] = best
        print(f"  reps={r}: best wall {best*1e3:.1f} ms")
    return (t[r_hi] - t[1]) / (r_hi - 1) * 1e9


def kernel_traced(**inputs):
    """Like kernel() but with NTFF tracing when available."""
    flags, in_maps = _host_prep(inputs)
    nc = _get_nc(flags)
    try:
        res = run_bass_kernel_spmd(nc, in_maps, list(range(NCORES)), trace=True)
    except ModuleNotFoundError:
        res = run_bass_kernel_spmd(nc, in_maps, list(range(NCORES)))
    out = np.concatenate([res.results[c]["out"] for c in range(NCORES)], axis=0)
    return out, res



# revision 13
# speedup vs baseline: 1.1730x; 1.1730x over previous
"""MLP-Mixer forward on 8 Trainium2 NeuronCores, data-parallel over batch.

Strategy
--------
Pure data parallel: 64 samples -> 8 per core; all parameters replicated.
Per core, the 8 residual states h[s] (427x427 fp32) stay resident in SBUF
across all 8 mixer blocks; block weights stream from HBM into
double-buffered SBUF slots.

All mixer matmuls run as fp8-e4m3 DoubleRow ("3-term hi/lo split"):
every operand X is split into X_hi = fp8(X) and X_lo = fp8(X - X_hi);
the product uses Wh@Ah + Wl@Ah + Wh@Al (the lo*lo term is dropped,
~1e-4 relative).  Each DoubleRow pass contracts a k-PAIR (2x128 rows)
at 0.5 cyc/row, so a logical K-tile costs 1.5 passes x 0.5 = 0.75x the
bf16 rate while keeping bf16-level accuracy (verified 4.4e-3 vs the
2e-2 budget in an exact numpy emulation; every single-fp8 operand
variant measured >3e-2 and was rejected).

Weights are host-split (hi/lo at scale 64, K padded 427->512 with zeros
so the garbage rows of the 4th activation k-plane are annihilated).
Activations split on-chip: LN writes a bf16 scratch t, Pool copies
t->hi(fp8), DVE subtracts t-hi->lo(fp8).  gelu writes a bf16 pair tile
(ACT, descale 1/64 folded into the activation scale), split per PAIR on
Pool+alternating DVE/Pool.  The residual add descales by 1/64 via
scalar_tensor_tensor on the Pool engine (h += acc/64).

Layouts per sample (row tiles = 4 chunks of 128|43 partitions):
  h    [128, 4, 428] fp32      residual stream
  t    [128, 4, 428] bf16      LN scratch (also the final-stage f1)
  hn_h/hn_l [128, 4, 432] fp8  split LN1(h), k-planes for MM1 pairs
  gy pairs  [128, 2, 432] fp8  split gelu out, planes = MM2 k-pairs
  token: yT = w1.T @ hn (m=TOK tiles); zT = w2.T @ gy -> h-layout
  channel: t -> PE-transpose(bf16) -> split -> uT = c1.T @ ytT;
           MM2 swapped (gy stationary, c2 moving) -> h-layout
  final stage: all-bf16 (f1T transpose + lw matmul + LNf/abs/mask);
  fp8 would cost ~4e-2 error here (no residual dilution), bf16 is
  only ~3% of the matmul budget.

LayerNorm stats: single-pass DVE bn_stats/bn_aggr + Newton sqrt (keeps
ScalarE on the gelu table).  Scheduling: the baseline software pipeline
is preserved (prep two samples ahead, MM2 lagging MM1 by one pair,
final stage zipped into the last channel phase).
"""

import os
from contextlib import ExitStack

import numpy as np
import ml_dtypes

import concourse.bacc as bacc
import concourse.tile as tile
import concourse.mybir as mybir
from concourse.bass import ts
from concourse.bass_utils import run_bass_kernel_spmd
from concourse.masks import make_identity

B, C, DIM, DEPTH, TOK = 64, 3, 427, 8, 2048
NCORES = 8
SPC = B // NCORES           # samples per core
P = 128
WP = DIM + 1                # 428, bf16/f32 tile stride
W8 = 432                    # fp8 tile stride (16-byte aligned)
NT = TOK // P               # 16
NPAIR = NT // 2             # 8 k-pairs for the second matmul
RT = [(0, 128), (128, 128), (256, 128), (384, 43)]  # DIM row/k tiles
NR = len(RT)
EPS = 1e-5
WS = 64.0                   # weight scale before fp8 quantization
F32 = mybir.dt.float32
BF16 = mybir.dt.bfloat16
FP8 = mybir.dt.float8e4
AF = mybir.ActivationFunctionType
ALU = mybir.AluOpType
DR = mybir.MatmulPerfMode.DoubleRow

_BUILD_CACHE = {}


def _build(ln1_triv, ln2_triv, lnf_triv, lb_triv, b1_triv, b2_triv,
           reps=1):
    nc = bacc.Bacc("TRN2", target_bir_lowering=False, debug=False,
                   num_devices=NCORES)

    x_d = nc.dram_tensor("x", [SPC, C, DIM, DIM], F32, kind="ExternalInput").ap()
    # fp8 hi/lo weight pairs; first-matmul weights K-padded 427->512 (zeros)
    rw1h_d = nc.dram_tensor("rw1h", [DEPTH, NR, P, TOK], FP8, kind="ExternalInput").ap()
    rw1l_d = nc.dram_tensor("rw1l", [DEPTH, NR, P, TOK], FP8, kind="ExternalInput").ap()
    rw2h_d = nc.dram_tensor("rw2h", [DEPTH, NT, P, DIM], FP8, kind="ExternalInput").ap()
    rw2l_d = nc.dram_tensor("rw2l", [DEPTH, NT, P, DIM], FP8, kind="ExternalInput").ap()
    cw1h_d = nc.dram_tensor("cw1h", [DEPTH, NR, P, TOK], FP8, kind="ExternalInput").ap()
    cw1l_d = nc.dram_tensor("cw1l", [DEPTH, NR, P, TOK], FP8, kind="ExternalInput").ap()
    cw2h_d = nc.dram_tensor("cw2h", [DEPTH, NT, P, DIM], FP8, kind="ExternalInput").ap()
    cw2l_d = nc.dram_tensor("cw2l", [DEPTH, NT, P, DIM], FP8, kind="ExternalInput").ap()
    lw_d = nc.dram_tensor("lw", [DIM, DIM], BF16, kind="ExternalInput").ap()
    dmask_d = nc.dram_tensor("dmask", [NR, P, DIM], F32, kind="ExternalInput").ap()
    rb1_d = rb2_d = cb1_d = cb2b_d = None
    if not b1_triv:
        rb1_d = nc.dram_tensor("rb1", [DEPTH, TOK], F32, kind="ExternalInput").ap()
        cb1_d = nc.dram_tensor("cb1", [DEPTH, TOK], F32, kind="ExternalInput").ap()
    if not b2_triv:
        rb2_d = nc.dram_tensor("rb2", [DEPTH, DIM], F32, kind="ExternalInput").ap()
        cb2b_d = nc.dram_tensor("cb2b", [DEPTH, P, DIM], F32, kind="ExternalInput").ap()
    ln1g_d = ln2g_d = lnfg_d = lb_d = None
    if not ln1_triv:
        ln1g_d = nc.dram_tensor("ln1gb", [DEPTH, 2, P, DIM], F32, kind="ExternalInput").ap()
    if not ln2_triv:
        ln2g_d = nc.dram_tensor("ln2gb", [DEPTH, 2, P, DIM], F32, kind="ExternalInput").ap()
    if not lnf_triv:
        lnfg_d = nc.dram_tensor("lnfgb", [2, P, DIM], F32, kind="ExternalInput").ap()
    if not lb_triv:
        lb_d = nc.dram_tensor("lbb", [P, DIM], F32, kind="ExternalInput").ap()
    out_d = nc.dram_tensor("out", [SPC, DIM, DIM], F32, kind="ExternalOutput").ap()

    with tile.TileContext(nc) as tc, ExitStack() as ctx:
        hp = ctx.enter_context(tc.tile_pool(name="h", bufs=SPC))
        wa = ctx.enter_context(tc.tile_pool(name="wa", bufs=2))
        wb = ctx.enter_context(tc.tile_pool(name="wb", bufs=2))
        tsp = ctx.enter_context(tc.tile_pool(name="tscr", bufs=4))
        ttp = ctx.enter_context(tc.tile_pool(name="tT", bufs=4))
        hnp = ctx.enter_context(tc.tile_pool(name="hn8", bufs=3))
        ytp = ctx.enter_context(tc.tile_pool(name="yt8", bufs=3))
        gyfp = ctx.enter_context(tc.tile_pool(name="gyf", bufs=3))
        gyp8 = ctx.enter_context(tc.tile_pool(name="gy8", bufs=5))
        vbp = ctx.enter_context(tc.tile_pool(name="vb", bufs=1))
        otp = ctx.enter_context(tc.tile_pool(name="ot", bufs=1))
        sqp = ctx.enter_context(tc.tile_pool(name="sq", bufs=1))
        xpp = ctx.enter_context(tc.tile_pool(name="xp", bufs=1))
        cst = ctx.enter_context(tc.tile_pool(name="cst", bufs=1))
        bp = ctx.enter_context(tc.tile_pool(name="bias", bufs=2))
        stp = ctx.enter_context(tc.tile_pool(name="st", bufs=2))
        gbp = ctx.enter_context(tc.tile_pool(name="gb", bufs=2))
        ps_mo = ctx.enter_context(tc.tile_pool(name="mo", bufs=2, space="PSUM"))
        ps_acc = ctx.enter_context(tc.tile_pool(name="acc", bufs=4, space="PSUM"))
        ps_tp = ctx.enter_context(tc.tile_pool(name="tp", bufs=2, space="PSUM"))

        # persistent tiles
        h_t = [hp.tile([P, NR, WP], F32, tag="h", name=f"h{i}") for i in range(SPC)]
        ident = cst.tile([P, P], BF16, tag="ident")
        make_identity(nc, ident[:, :])
        dm_t = cst.tile([P, NR, DIM], F32, tag="dmask")
        dm_loaded = [False]

        def load_dmask():
            if not dm_loaded[0]:
                dm_loaded[0] = True
                for r in range(NR):
                    nc.sync.dma_start(dm_t[:, r, :], dmask_d[r])

        def ln_stats(srcs, want_nmr=False):
            """srcs: list of NR APs [rsz, DIM]. Returns (mv, rstd[, nmr])
            as [P, NR] tiles (column r = row-tile r)."""
            st6 = stp.tile([P, NR, 6], F32, tag="st6")
            mv = stp.tile([P, NR, 2], F32, tag="mv")
            for r, src in enumerate(srcs):
                rsz = RT[r][1]
                nc.vector.bn_stats(st6[:rsz, r, :], src)
                nc.vector.bn_aggr(mv[:rsz, r, :], st6[:rsz, r, :])
            var = stp.tile([P, NR], F32, tag="var")
            y = stp.tile([P, NR], F32, tag="nwy")
            q = stp.tile([P, NR], F32, tag="nwq")
            rstd = stp.tile([P, NR], F32, tag="rstd")
            nc.vector.tensor_scalar(var[:, :], mv[:, :, 1], EPS, None, ALU.add)
            # Newton sqrt: y0 = 0.5*(1+v); y <- 0.5*(y + v/y)  (4 iters)
            nc.vector.tensor_scalar(y[:, :], var[:, :], 1.0, 0.5, ALU.add, ALU.mult)
            for _ in range(4):
                nc.vector.reciprocal(q[:, :], y[:, :])
                nc.vector.tensor_tensor(q[:, :], var[:, :], q[:, :], ALU.mult)
                nc.vector.tensor_tensor(y[:, :], y[:, :], q[:, :], ALU.add)
                nc.vector.tensor_scalar(y[:, :], y[:, :], 0.5, None, ALU.mult)
            nc.vector.reciprocal(rstd[:, :], y[:, :])
            if want_nmr:
                nmr = stp.tile([P, NR], F32, tag="nmr")
                nc.vector.tensor_tensor(nmr[:, :], mv[:, :, 0], rstd[:, :],
                                        ALU.mult)
                nc.vector.tensor_scalar(nmr[:, :], nmr[:, :], -1.0, None,
                                        ALU.mult)
                return mv, rstd, nmr
            return mv, rstd

        def ln_apply(srcs, dst, mv, rstd, gb_tile):
            """dst[:rsz, r, :DIM] = LN of srcs[r] (bf16)."""
            for r in range(NR):
                rsz = RT[r][1]
                nc.vector.tensor_scalar(
                    dst[:rsz, r, :DIM], srcs[r], mv[:rsz, r, 0:1],
                    rstd[:rsz, r:r + 1], ALU.subtract, ALU.mult)
                if gb_tile is not None:
                    nc.vector.tensor_tensor(
                        dst[:rsz, r, :DIM], dst[:rsz, r, :DIM],
                        gb_tile[:rsz, 0, :DIM], ALU.mult)
                    nc.vector.tensor_tensor(
                        dst[:rsz, r, :DIM], dst[:rsz, r, :DIM],
                        gb_tile[:rsz, 1, :DIM], ALU.add)

        def split_hilo(t, hi, lo):
            """hi = fp8(t), lo = fp8(t - hi) per row tile; zero the pad rows
            of the 4th k-plane so DoubleRow k-pair reads see 0, not junk.
            hi + lo on GPSIMD, memsets on GPSIMD (frees DVE for gy-lo)."""
            nc.gpsimd.memset(hi[:, NR - 1, :], 0.0)
            nc.gpsimd.memset(lo[:, NR - 1, :], 0.0)
            for r in range(NR):
                rsz = RT[r][1]
                nc.gpsimd.tensor_copy(hi[:rsz, r, :DIM], t[:rsz, r, :DIM])
                nc.gpsimd.tensor_tensor(lo[:rsz, r, :DIM], t[:rsz, r, :DIM],
                                        hi[:rsz, r, :DIM], ALU.subtract)

        def load_gb(dram_ap, tag):
            t = gbp.tile([P, 2, DIM], F32, tag=tag)
            nc.sync.dma_start(t[:, 0, :], dram_ap[0])
            nc.sync.dma_start(t[:, 1, :], dram_ap[1])
            return t

        def h_rows(s):
            return [h_t[s][:RT[r][1], r, :DIM] for r in range(NR)]

        def xprep(s):
            for r in range(NR):
                r0, rsz = RT[r]
                xs = []
                for c in range(C):
                    xt = xpp.tile([P, WP], F32, tag=f"xp{c}")
                    nc.sync.dma_start(xt[:rsz, :DIM], x_d[s, c, r0:r0 + rsz, :])
                    xs.append(xt)
                hr = h_t[s][:rsz, r, :DIM]
                nc.vector.tensor_tensor(hr, xs[0][:rsz, :DIM], xs[1][:rsz, :DIM], ALU.add)
                nc.vector.tensor_tensor(hr, hr, xs[2][:rsz, :DIM], ALU.add)
                nc.vector.tensor_scalar(hr, hr, 1.0 / C, None, ALU.mult)

        def load_w1pair(hi_ap, lo_ap):
            """[NR, P, TOK] fp8 -> [P, NR, TOK] hi+lo, chunked for streaming."""
            th = wa.tile([P, NR, TOK], FP8, tag="w1h")
            tl = wa.tile([P, NR, TOK], FP8, tag="w1l")
            CH_ = 512
            for m0 in range(0, TOK, CH_):
                for k in range(NR):
                    nc.sync.dma_start(th[:, k, m0:m0 + CH_], hi_ap[k, :, m0:m0 + CH_])
                    nc.sync.dma_start(tl[:, k, m0:m0 + CH_], lo_ap[k, :, m0:m0 + CH_])
            return th, tl

        def load_w2pair(hi_ap, lo_ap):
            """[NT, P, DIM] fp8 -> [P, NT, W8] hi+lo."""
            th = wb.tile([P, NT, W8], FP8, tag="w2h")
            tl = wb.tile([P, NT, W8], FP8, tag="w2l")
            for k in range(NT):
                nc.sync.dma_start(th[:, k, :DIM], hi_ap[k])
                nc.sync.dma_start(tl[:, k, :DIM], lo_ap[k])
            return th, tl

        def load_b1like(dram_ap):
            t = bp.tile([P, NT], F32, tag="b1")
            nc.sync.dma_start(t[:, :], dram_ap.rearrange("(k p) -> p k", p=P))
            return t

        def load_b2like(dram_ap):
            t = bp.tile([P, NR], F32, tag="b2")
            nc.sync.dma_start(t[:, :3], dram_ap[:384].rearrange("(d p) -> p d", p=P))
            nc.sync.dma_start(t[:43, 3:4], dram_ap[384:, None])
            return t

        def mixer_mms(hh, hl, w1h, w1l, w2h, w2l, b1t, swap2):
            """3-term fp8 DoubleRow chain: per m: MM1 (6 DR passes over 2
            k-pairs x 3 hi/lo terms); gelu -> bf16 pair tile; split pair to
            fp8 hi/lo; MM2 per k-pair j: 3 DR passes x 4 DIM-chunks.
            swap2: MM2 uses the gy pair as STATIONARY and w2 rows MOVING so
            the out lands in h-layout (channel mixer)."""
            accs = [ps_acc.tile([P, WP], F32, tag="acc", name=f"acc{i}") for i in range(NR)]

            def mm2_pair(j, gh, gl):
                first, last = (j == 0), (j == NPAIR - 1)
                for d in range(NR):
                    d0, dsz = RT[d]
                    if swap2:
                        terms = ((gh[:, 0:2, d0:d0 + dsz], w2h[:, 2 * j:2 * j + 2, :DIM]),
                                 (gl[:, 0:2, d0:d0 + dsz], w2h[:, 2 * j:2 * j + 2, :DIM]),
                                 (gh[:, 0:2, d0:d0 + dsz], w2l[:, 2 * j:2 * j + 2, :DIM]))
                    else:
                        terms = ((w2h[:, 2 * j:2 * j + 2, d0:d0 + dsz], gh[:, 0:2, :DIM]),
                                 (w2l[:, 2 * j:2 * j + 2, d0:d0 + dsz], gh[:, 0:2, :DIM]),
                                 (w2h[:, 2 * j:2 * j + 2, d0:d0 + dsz], gl[:, 0:2, :DIM]))
                    for ti, (S, Mv) in enumerate(terms):
                        nc.tensor.matmul(accs[d][:dsz, :DIM], S, Mv,
                                         start=(first and ti == 0),
                                         stop=(last and ti == 2),
                                         perf_mode=DR)

            pend = []          # completed pairs awaiting MM2 (lag 2 pairs)
            gf = gh = gl = None
            for m in range(NT):
                if m % 2 == 0:
                    gf = gyfp.tile([P, 2, WP], BF16, tag="gyf")
                    gh = gyp8.tile([P, 2, W8], FP8, tag="gyh")
                    gl = gyp8.tile([P, 2, W8], FP8, tag="gyl")
                y_ps = ps_mo.tile([P, WP], F32, tag="mo")
                for kp in range(2):
                    k2 = slice(2 * kp, 2 * kp + 2)
                    terms = ((w1h[:, k2, ts(m, P)], hh[:, k2, :DIM]),
                             (w1l[:, k2, ts(m, P)], hh[:, k2, :DIM]),
                             (w1h[:, k2, ts(m, P)], hl[:, k2, :DIM]))
                    for ti, (S, Mv) in enumerate(terms):
                        nc.tensor.matmul(y_ps[:, :DIM], S, Mv,
                                         start=(kp == 0 and ti == 0),
                                         stop=(kp == 1 and ti == 2),
                                         perf_mode=DR)
                if b1t is None:
                    nc.scalar.activation(gf[:, m % 2, :DIM], y_ps[:, :DIM],
                                         AF.Gelu, scale=1.0 / WS)
                else:
                    nc.scalar.activation(gf[:, m % 2, :DIM], y_ps[:, :DIM],
                                         AF.Gelu, scale=1.0 / WS,
                                         bias=b1t[:, m:m + 1])
                if m % 2 == 1:
                    j = m // 2
                    # pair split: hi = fp8(gf) on ACT (one 854-wide copy),
                    # lo = gf - hi on GPSIMD
                    nc.scalar.activation(gh[:, 0:2, :DIM], gf[:, 0:2, :DIM],
                                         AF.Identity, bias=0.0)
                    nc.vector.tensor_tensor(gl[:, 0:2, :DIM], gf[:, 0:2, :DIM],
                                            gh[:, 0:2, :DIM], ALU.subtract)
                    pend.append((j, gh, gl))
                    if len(pend) > 3:
                        mm2_pair(*pend.pop(0))
            for p_ in pend:
                mm2_pair(*p_)
            return accs

        def transpose_to(dst, src_tile):
            """PE-transpose src_tile (bf16 row-tiles) into dst [P, NR, WP]."""
            for c in range(NR):
                c0, csz = RT[c]
                tp = ps_tp.tile([P, WP], BF16, tag="tp")
                for r in range(NR):
                    r0, rsz = RT[r]
                    nc.tensor.transpose(tp[:csz, r0:r0 + rsz],
                                        src_tile[:rsz, r, c0:c0 + csz],
                                        ident[:rsz, :rsz])
                nc.vector.tensor_copy(dst[:csz, c, :DIM], tp[:csz, :DIM])

        def add_resid(s, accs, b2t, d2bt):
            """h[s] += accs/WS (+ biases), on DVE."""
            for d in range(NR):
                dsz = RT[d][1]
                hr = h_t[s][:dsz, d, :DIM]
                nc.vector.scalar_tensor_tensor(hr, accs[d][:dsz, :DIM],
                                               1.0 / WS, hr, ALU.mult, ALU.add)
                if b2t is not None:
                    nc.vector.tensor_scalar_add(hr, hr, b2t[:dsz, d:d + 1])
                if d2bt is not None:
                    nc.vector.tensor_tensor(hr, hr, d2bt[:dsz, :DIM], ALU.add)

        # ---------------- main program ----------------
        depth = int(os.environ.get("KMIX_DEPTH", DEPTH))

        class _Ph:
            pass

        def tok_phase(blk):
            ph = _Ph()

            def load_small():
                ph.b1 = None if b1_triv else load_b1like(rb1_d[blk])
                ph.b2 = None if b2_triv else load_b2like(rb2_d[blk])
                ph.gb = None if ln1_triv else load_gb(ln1g_d[blk], "gb1")

            def load_big():
                ph.w1 = load_w1pair(rw1h_d[blk], rw1l_d[blk])
                ph.w2 = load_w2pair(rw2h_d[blk], rw2l_d[blk])

            def prep(s):
                if blk == 0:
                    xprep(s)
                mn, rstd = ln_stats(h_rows(s))
                t = tsp.tile([P, NR, WP], BF16, tag="t")
                ln_apply(h_rows(s), t, mn, rstd, ph.gb)
                hh = hnp.tile([P, NR, W8], FP8, tag="hnh")
                hl = hnp.tile([P, NR, W8], FP8, tag="hnl")
                split_hilo(t, hh, hl)
                return (hh, hl)

            def mm(s, handle):
                return mixer_mms(handle[0], handle[1], *ph.w1, *ph.w2,
                                 ph.b1, swap2=False)

            def resid(s, accs):
                add_resid(s, accs, ph.b2, None)

            ph.load_small, ph.load_big = load_small, load_big
            ph.prep, ph.mm, ph.resid = prep, mm, resid
            ph.kind = "tok"
            return ph

        def ch_phase(blk):
            ph = _Ph()

            def load_small():
                ph.b1 = None if b1_triv else load_b1like(cb1_d[blk])
                ph.d2 = None
                if not b2_triv:
                    ph.d2 = gbp.tile([P, DIM], F32, tag="d2b")
                    nc.sync.dma_start(ph.d2[:, :], cb2b_d[blk])
                ph.gb = None if ln2_triv else load_gb(ln2g_d[blk], "gb2")

            def load_big():
                ph.w1 = load_w1pair(cw1h_d[blk], cw1l_d[blk])
                ph.w2 = load_w2pair(cw2h_d[blk], cw2l_d[blk])

            def prep(s):
                mn, rstd = ln_stats(h_rows(s))
                t = tsp.tile([P, NR, WP], BF16, tag="t")
                ln_apply(h_rows(s), t, mn, rstd, ph.gb)
                tT = ttp.tile([P, NR, WP], BF16, tag="tT")
                transpose_to(tT, t)
                yh = ytp.tile([P, NR, W8], FP8, tag="yth")
                yl = ytp.tile([P, NR, W8], FP8, tag="ytl")
                split_hilo(tT, yh, yl)
                return (yh, yl)

            def mm(s, handle):
                return mixer_mms(handle[0], handle[1], *ph.w1, *ph.w2,
                                 ph.b1, swap2=True)

            def resid(s, accs):
                add_resid(s, accs, None, ph.d2)

            ph.load_small, ph.load_big = load_small, load_big
            ph.prep, ph.mm, ph.resid = prep, mm, resid
            ph.kind = "ch"
            return ph

        def fin_phase():
            ph = _Ph()

            def load_small():
                load_dmask()
                ph.gbf = None if lnf_triv else load_gb(lnfg_d, "gbf")
                ph.lb = None
                if not lb_triv:
                    ph.lb = gbp.tile([P, DIM], F32, tag="lbb")
                    nc.sync.dma_start(ph.lb[:, :], lb_d)

            def load_big():
                ph.lw = cst.tile([P, NR, WP], BF16, tag="lwf")
                for k in range(NR):
                    k0, ksz = RT[k]
                    nc.sync.dma_start(ph.lw[:ksz, k, :DIM], lw_d[k0:k0 + ksz, :])

            def prep(s):
                if depth == 0:      # debug path: no mixer blocks ran
                    xprep(s)
                mn, rstd = ln_stats(h_rows(s))
                f1 = tsp.tile([P, NR, WP], BF16, tag="t")
                ln_apply(h_rows(s), f1, mn, rstd, ph.gbf)
                f1T = ttp.tile([P, NR, WP], BF16, tag="tT")
                transpose_to(f1T, f1)
                return f1T

            def mm(s, f1T):
                f2 = vbp.tile([P, NR, WP], F32, tag="vb")
                for m in range(NR):
                    m0, msz = RT[m]
                    acc3 = ps_mo.tile([P, WP], F32, tag="mo")
                    for c in range(NR):
                        csz = RT[c][1]
                        nc.tensor.matmul(acc3[:msz, :DIM],
                                         f1T[:csz, c, m0:m0 + msz],
                                         ph.lw[:csz, c, :DIM],
                                         start=(c == 0), stop=(c == NR - 1))
                    nc.scalar.activation(f2[:msz, m, :DIM], acc3[:msz, :DIM],
                                         AF.Identity, bias=0.0)
                    if ph.lb is not None:
                        nc.vector.tensor_tensor(f2[:msz, m, :DIM],
                                                f2[:msz, m, :DIM],
                                                ph.lb[:msz, :DIM], ALU.add)
                f2rows = [f2[:RT[r][1], r, :DIM] for r in range(NR)]
                mn2, rstd2, nmr2 = ln_stats(f2rows, want_nmr=True)
                ot = otp.tile([P, NR, WP], F32, tag="ot")
                for r in range(NR):
                    r0, rsz = RT[r]
                    if ph.gbf is None:
                        nc.scalar.activation(ot[:rsz, r, :DIM], f2rows[r], AF.Abs,
                                             scale=rstd2[:rsz, r:r + 1],
                                             bias=nmr2[:rsz, r:r + 1])
                    else:
                        nc.vector.tensor_scalar(ot[:rsz, r, :DIM], f2rows[r],
                                                mn2[:rsz, r, 0:1],
                                                rstd2[:rsz, r:r + 1],
                                                ALU.subtract, ALU.mult)
                        nc.vector.tensor_tensor(ot[:rsz, r, :DIM],
                                                ot[:rsz, r, :DIM],
                                                ph.gbf[:rsz, 0, :DIM], ALU.mult)
                        nc.vector.tensor_tensor(ot[:rsz, r, :DIM],
                                                ot[:rsz, r, :DIM],
                                                ph.gbf[:rsz, 1, :DIM], ALU.add)
                        nc.scalar.activation(ot[:rsz, r, :DIM],
                                             ot[:rsz, r, :DIM], AF.Abs)
                    nc.vector.tensor_tensor(ot[:rsz, r, :DIM], ot[:rsz, r, :DIM],
                                            dm_t[:rsz, r, :DIM], ALU.mult)
                    nc.sync.dma_start(out_d[s, r0:r0 + rsz, :],
                                      ot[:rsz, r, :DIM])
                return None

            def resid(s, accs):
                pass

            ph.load_small, ph.load_big = load_small, load_big
            ph.prep, ph.mm, ph.resid = prep, mm, resid
            ph.kind = "fin"
            return ph

        phases = []
        for rep in range(reps):
            for blk in range(depth):
                phases.append(tok_phase(blk))
                phases.append(ch_phase(blk))
            phases.append(fin_phase())

        steps = []
        for pi, ph in enumerate(phases):
            if ph.kind == "fin" and pi > 0 and phases[pi - 1].kind == "ch":
                phases[pi - 1].zipf = ph
                continue
            for s in range(SPC):
                steps.append((ph, s))

        handles = {}

        def ensure_loaded(j):
            """Kick off a phase's weight DMAs well before its first step so
            the loads hide under the previous phase's (fast fp8) compute."""
            if j >= len(steps):
                return
            ph = steps[j][0]
            if not getattr(ph, "loaded", False):
                ph.load_small()
                ph.load_big()
                ph.loaded = True

        def emit_prep(j):
            if j >= len(steps):
                return
            ph, s = steps[j]
            handles[j] = ph.prep(s)

        ensure_loaded(0)
        for jj in range(1, 7):
            ensure_loaded(jj)
        emit_prep(0)
        emit_prep(1)
        pend = None        # (resid_fn, s, accs) awaiting residual add
        for j, (ph, s) in enumerate(steps):
            ensure_loaded(j + 6)
            emit_prep(j + 2)
            zipf = getattr(ph, "zipf", None)
            fh = None
            if zipf is not None and s >= 2:
                if not getattr(zipf, "loaded", False):
                    zipf.load_small()
                    zipf.load_big()
                    zipf.loaded = True
                fh = zipf.prep(s - 2)
            if pend is not None:
                pend[0](pend[1], pend[2])
                pend = None
            accs = ph.mm(s, handles.pop(j))
            if accs is not None:
                pend = (ph.resid, s, accs)
            if fh is not None:
                zipf.mm(s - 2, fh)
            if zipf is not None and s == SPC - 1:
                if pend is not None:
                    pend[0](pend[1], pend[2])
                    pend = None
                for s2 in (SPC - 2, SPC - 1):
                    fh = zipf.prep(s2)
                    zipf.mm(s2, fh)
        if pend is not None:
            pend[0](pend[1], pend[2])

    nc.compile()
    return nc


def _host_prep(inputs):
    g = {k: np.asarray(v, dtype=np.float32) for k, v in inputs.items()}
    ln1_triv = bool(np.all(g["ln1_g"] == 1.0) and np.all(g["ln1_b"] == 0.0))
    ln2_triv = bool(np.all(g["ln2_g"] == 1.0) and np.all(g["ln2_b"] == 0.0))
    lnf_triv = bool(np.all(g["lnf_g"] == 1.0) and np.all(g["lnf_b"] == 0.0))
    lb_triv = bool(np.all(g["lb"] == 0.0))
    b1_triv = bool(np.all(g["rb1"] == 0.0) and np.all(g["cb1"] == 0.0))
    b2_triv = bool(np.all(g["rb2"] == 0.0) and np.all(g["cb2"] == 0.0))
    flags = (ln1_triv, ln2_triv, lnf_triv, lb_triv, b1_triv, b2_triv)

    dmask = np.ones((NR, P, DIM), dtype=np.float32)
    for r in range(NR):
        for p in range(min(P, DIM - 128 * r)):
            dmask[r, p, 128 * r + p] = 0.0

    def bcast2(gv, bv):
        gb = np.stack([gv, bv], axis=-2)[..., None, :]
        return np.broadcast_to(gb, gb.shape[:-2] + (P, DIM)).copy()

    bf = ml_dtypes.bfloat16
    f8 = ml_dtypes.float8_e4m3

    def split_w1(w):  # [DEPTH, DIM, TOK] -> hi/lo [DEPTH, NR, P, TOK]
        wp = np.zeros((DEPTH, NR * P, TOK), np.float32)
        wp[:, :DIM] = w * WS
        hi = wp.astype(f8)
        lo = (wp - hi.astype(np.float32)).astype(f8)
        return (np.ascontiguousarray(hi.reshape(DEPTH, NR, P, TOK)),
                np.ascontiguousarray(lo.reshape(DEPTH, NR, P, TOK)))

    def split_w2(w):  # [DEPTH, TOK, DIM] -> hi/lo [DEPTH, NT, P, DIM]
        ws = w * WS
        hi = ws.astype(f8)
        lo = (ws - hi.astype(np.float32)).astype(f8)
        return (np.ascontiguousarray(hi.reshape(DEPTH, NT, P, DIM)),
                np.ascontiguousarray(lo.reshape(DEPTH, NT, P, DIM)))

    rw1h, rw1l = split_w1(g["rw1"])
    rw2h, rw2l = split_w2(g["rw2"])
    cw1h, cw1l = split_w1(g["cw1"])
    cw2h, cw2l = split_w2(g["cw2"])
    common = {
        "rw1h": rw1h, "rw1l": rw1l, "rw2h": rw2h, "rw2l": rw2l,
        "cw1h": cw1h, "cw1l": cw1l, "cw2h": cw2h, "cw2l": cw2l,
        "lw": np.ascontiguousarray(g["lw"].astype(bf)),
        "dmask": dmask,
    }
    if not b1_triv:
        common["rb1"] = g["rb1"]
        common["cb1"] = g["cb1"]
    if not b2_triv:
        common["rb2"] = g["rb2"]
        common["cb2b"] = np.ascontiguousarray(
            np.broadcast_to(g["cb2"][:, None, :], (DEPTH, P, DIM)))
    if not ln1_triv:
        common["ln1gb"] = np.ascontiguousarray(bcast2(g["ln1_g"], g["ln1_b"]))
    if not ln2_triv:
        common["ln2gb"] = np.ascontiguousarray(bcast2(g["ln2_g"], g["ln2_b"]))
    if not lnf_triv:
        common["lnfgb"] = np.ascontiguousarray(bcast2(g["lnf_g"], g["lnf_b"]))
    if not lb_triv:
        common["lbb"] = np.broadcast_to(g["lb"][None, :], (P, DIM)).copy()
    x = np.ascontiguousarray(g["x"])
    in_maps = [dict(common, x=np.ascontiguousarray(x[c * SPC:(c + 1) * SPC]))
               for c in range(NCORES)]
    return flags, in_maps


def _get_nc(flags, reps=1):
    key = flags + (reps,)
    if key not in _BUILD_CACHE:
        _BUILD_CACHE[key] = _build(*flags, reps=reps)
    return _BUILD_CACHE[key]


def kernel(**inputs):
    flags, in_maps = _host_prep(inputs)
    nc = _get_nc(flags)
    res = run_bass_kernel_spmd(nc, in_maps, list(range(NCORES)))
    return np.concatenate([res.results[c]["out"] for c in range(NCORES)], axis=0)


def measure_hw_time(inputs, r_hi=4, iters=5):
    """Wall-clock repetition-slope timing on the real device."""
    import time as _time
    flags, in_maps = _host_prep(inputs)
    cores = list(range(NCORES))
    t = {}
    for r in (1, r_hi):
        nc = _get_nc(flags, reps=r)
        best = float("inf")
        run_bass_kernel_spmd(nc, in_maps, cores)  # warm (jit + neff cache)
        for _ in range(iters):
            t0 = _time.perf_counter()
            run_bass_kernel_spmd(nc, in_maps, cores)
            best = min(best, _time.perf_counter() - t0)
        t[r] = best
        print(f"  reps={r}: best wall {best*1e3:.1f} ms")
    return (t[r_hi] - t[1]) / (r_hi - 1) * 1e9


def kernel_traced(**inputs):
    """Like kernel() but with NTFF tracing when available."""
    flags, in_maps = _host_prep(inputs)
    nc = _get_nc(flags)
    try:
        res = run_bass_kernel_spmd(nc, in_maps, list(range(NCORES)), trace=True)
    except ModuleNotFoundError:
        res = run_bass_kernel_spmd(nc, in_maps, list(range(NCORES)))
    out = np.concatenate([res.results[c]["out"] for c in range(NCORES)], axis=0)
    return out, res


# revision 30
# speedup vs baseline: 1.2368x; 1.0544x over previous
"""MLP-Mixer forward on 8 Trainium2 NeuronCores, data-parallel over batch.

Strategy
--------
Pure data parallel: 64 samples -> 8 per core; all parameters replicated.
Per core, the 8 residual states h[s] (427x427 fp32) stay resident in SBUF
across all 8 mixer blocks; block weights stream from HBM into
double-buffered SBUF slots.

All mixer matmuls run as fp8-e4m3 DoubleRow ("3-term hi/lo split"):
every operand X is split into X_hi = fp8(X) and X_lo = fp8(X - X_hi);
the product uses Wh@Ah + Wl@Ah + Wh@Al (the lo*lo term is dropped,
~1e-4 relative).  Each DoubleRow pass contracts a k-PAIR (2x128 rows)
at 0.5 cyc/row, so a logical K-tile costs 1.5 passes x 0.5 = 0.75x the
bf16 rate while keeping bf16-level accuracy (verified 4.4e-3 vs the
2e-2 budget in an exact numpy emulation; every single-fp8 operand
variant measured >3e-2 and was rejected).

Weights are host-split (hi/lo at scale 64, K padded 427->512 with zeros
so the garbage rows of the 4th activation k-plane are annihilated).
Activations split on-chip: LN writes a bf16 scratch t, Pool copies
t->hi(fp8), DVE subtracts t-hi->lo(fp8).  gelu writes a bf16 pair tile
(ACT, descale 1/64 folded into the activation scale), split per PAIR on
Pool+alternating DVE/Pool.  The residual add descales by 1/64 via
scalar_tensor_tensor on the Pool engine (h += acc/64).

Layouts per sample (row tiles = 4 chunks of 128|43 partitions):
  h    [128, 4, 428] fp32      residual stream
  t    [128, 4, 428] bf16      LN scratch (also the final-stage f1)
  hn_h/hn_l [128, 4, 432] fp8  split LN1(h), k-planes for MM1 pairs
  gy pairs  [128, 2, 432] fp8  split gelu out, planes = MM2 k-pairs
  token: yT = w1.T @ hn (m=TOK tiles); zT = w2.T @ gy -> h-layout
  channel: t -> PE-transpose(bf16) -> split -> uT = c1.T @ ytT;
           MM2 swapped (gy stationary, c2 moving) -> h-layout
  final stage: all-bf16 (f1T transpose + lw matmul + LNf/abs/mask);
  fp8 would cost ~4e-2 error here (no residual dilution), bf16 is
  only ~3% of the matmul budget.

LayerNorm stats: single-pass DVE bn_stats/bn_aggr + Newton sqrt (keeps
ScalarE on the gelu table).  Scheduling: the baseline software pipeline
is preserved (prep two samples ahead, MM2 lagging MM1 by one pair,
final stage zipped into the last channel phase).
"""

import os
from contextlib import ExitStack

import numpy as np
import ml_dtypes

import concourse.bacc as bacc
import concourse.tile as tile
import concourse.mybir as mybir
from concourse.bass import ts
from concourse.bass_utils import run_bass_kernel_spmd
from concourse.masks import make_identity

B, C, DIM, DEPTH, TOK = 64, 3, 427, 8, 2048
NCORES = 8
SPC = B // NCORES           # samples per core
P = 128
WP = DIM + 1                # 428, bf16/f32 tile stride
W8 = 432                    # fp8 tile stride (16-byte aligned)
NT = TOK // P               # 16
NPAIR = NT // 2             # 8 k-pairs for the second matmul
RT = [(0, 128), (128, 128), (256, 128), (384, 43)]  # DIM row/k tiles
NR = len(RT)
EPS = 1e-5
WS = 64.0                   # weight scale before fp8 quantization
F32 = mybir.dt.float32
BF16 = mybir.dt.bfloat16
FP8 = mybir.dt.float8e4
AF = mybir.ActivationFunctionType
ALU = mybir.AluOpType
DR = mybir.MatmulPerfMode.DoubleRow

_BUILD_CACHE = {}


def _build(ln1_triv, ln2_triv, lnf_triv, lb_triv, b1_triv, b2_triv,
           reps=1):
    nc = bacc.Bacc("TRN2", target_bir_lowering=False, debug=False,
                   num_devices=NCORES)

    x_d = nc.dram_tensor("x", [SPC, C, DIM, DIM], F32, kind="ExternalInput").ap()
    # fp8 hi/lo weight pairs; first-matmul weights K-padded 427->512 (zeros)
    rw1h_d = nc.dram_tensor("rw1h", [DEPTH, NR, P, TOK], FP8, kind="ExternalInput").ap()
    rw1l_d = nc.dram_tensor("rw1l", [DEPTH, NR, P, TOK], FP8, kind="ExternalInput").ap()
    rw2h_d = nc.dram_tensor("rw2h", [DEPTH, NT, P, DIM], FP8, kind="ExternalInput").ap()
    rw2l_d = nc.dram_tensor("rw2l", [DEPTH, NT, P, DIM], FP8, kind="ExternalInput").ap()
    cw1h_d = nc.dram_tensor("cw1h", [DEPTH, NR, P, TOK], FP8, kind="ExternalInput").ap()
    cw1l_d = nc.dram_tensor("cw1l", [DEPTH, NR, P, TOK], FP8, kind="ExternalInput").ap()
    cw2h_d = nc.dram_tensor("cw2h", [DEPTH, NT, P, DIM], FP8, kind="ExternalInput").ap()
    cw2l_d = nc.dram_tensor("cw2l", [DEPTH, NT, P, DIM], FP8, kind="ExternalInput").ap()
    lw_d = nc.dram_tensor("lw", [DIM, DIM], BF16, kind="ExternalInput").ap()
    dmask_d = nc.dram_tensor("dmask", [NR, P, DIM], F32, kind="ExternalInput").ap()
    rb1_d = rb2_d = cb1_d = cb2b_d = None
    if not b1_triv:
        rb1_d = nc.dram_tensor("rb1", [DEPTH, TOK], F32, kind="ExternalInput").ap()
        cb1_d = nc.dram_tensor("cb1", [DEPTH, TOK], F32, kind="ExternalInput").ap()
    if not b2_triv:
        rb2_d = nc.dram_tensor("rb2", [DEPTH, DIM], F32, kind="ExternalInput").ap()
        cb2b_d = nc.dram_tensor("cb2b", [DEPTH, P, DIM], F32, kind="ExternalInput").ap()
    ln1g_d = ln2g_d = lnfg_d = lb_d = None
    if not ln1_triv:
        ln1g_d = nc.dram_tensor("ln1gb", [DEPTH, 2, P, DIM], F32, kind="ExternalInput").ap()
    if not ln2_triv:
        ln2g_d = nc.dram_tensor("ln2gb", [DEPTH, 2, P, DIM], F32, kind="ExternalInput").ap()
    if not lnf_triv:
        lnfg_d = nc.dram_tensor("lnfgb", [2, P, DIM], F32, kind="ExternalInput").ap()
    if not lb_triv:
        lb_d = nc.dram_tensor("lbb", [P, DIM], F32, kind="ExternalInput").ap()
    out_d = nc.dram_tensor("out", [SPC, DIM, DIM], F32, kind="ExternalOutput").ap()

    with tile.TileContext(nc) as tc, ExitStack() as ctx:
        hp = ctx.enter_context(tc.tile_pool(name="h", bufs=SPC))
        wa = ctx.enter_context(tc.tile_pool(name="wa", bufs=2))
        wb = ctx.enter_context(tc.tile_pool(name="wb", bufs=2))
        tsp = ctx.enter_context(tc.tile_pool(name="tscr", bufs=4))
        ttp = ctx.enter_context(tc.tile_pool(name="tT", bufs=4))
        hnp = ctx.enter_context(tc.tile_pool(name="hn8", bufs=3))
        ytp = ctx.enter_context(tc.tile_pool(name="yt8", bufs=3))
        gyfp = ctx.enter_context(tc.tile_pool(name="gyf", bufs=3))
        gyp8 = ctx.enter_context(tc.tile_pool(name="gy8", bufs=6))
        vbp = ctx.enter_context(tc.tile_pool(name="vb", bufs=1))
        otp = ctx.enter_context(tc.tile_pool(name="ot", bufs=1))
        sqp = ctx.enter_context(tc.tile_pool(name="sq", bufs=1))
        xpp = ctx.enter_context(tc.tile_pool(name="xp", bufs=1))
        cst = ctx.enter_context(tc.tile_pool(name="cst", bufs=1))
        bp = ctx.enter_context(tc.tile_pool(name="bias", bufs=2))
        stp = ctx.enter_context(tc.tile_pool(name="st", bufs=2))
        gbp = ctx.enter_context(tc.tile_pool(name="gb", bufs=2))
        ps_mo = ctx.enter_context(tc.tile_pool(name="mo", bufs=3, space="PSUM"))
        ps_acc = ctx.enter_context(tc.tile_pool(name="acc", bufs=4, space="PSUM"))
        ps_tp = ctx.enter_context(tc.tile_pool(name="tp", bufs=1, space="PSUM"))

        # persistent tiles
        h_t = [hp.tile([P, NR, WP], F32, tag="h", name=f"h{i}") for i in range(SPC)]
        ident = cst.tile([P, P], BF16, tag="ident")
        make_identity(nc, ident[:, :])
        dm_t = cst.tile([P, NR, DIM], F32, tag="dmask")
        dm_loaded = [False]

        def load_dmask():
            if not dm_loaded[0]:
                dm_loaded[0] = True
                for r in range(NR):
                    nc.sync.dma_start(dm_t[:, r, :], dmask_d[r])

        def ln_stats(srcs, want_nmr=False):
            """srcs: list of NR APs [rsz, DIM]. Returns (mv, rstd[, nmr])
            as [P, NR] tiles (column r = row-tile r)."""
            st6 = stp.tile([P, NR, 6], F32, tag="st6")
            mv = stp.tile([P, NR, 2], F32, tag="mv")
            for r, src in enumerate(srcs):
                rsz = RT[r][1]
                nc.vector.bn_stats(st6[:rsz, r, :], src)
                nc.vector.bn_aggr(mv[:rsz, r, :], st6[:rsz, r, :])
            var = stp.tile([P, NR], F32, tag="var")
            y = stp.tile([P, NR], F32, tag="nwy")
            q = stp.tile([P, NR], F32, tag="nwq")
            rstd = stp.tile([P, NR], F32, tag="rstd")
            nc.vector.tensor_scalar(var[:, :], mv[:, :, 1], EPS, None, ALU.add)
            # Newton sqrt: y0 = 0.5*(1+v); y <- 0.5*(y + v/y)  (4 iters)
            nc.vector.tensor_scalar(y[:, :], var[:, :], 1.0, 0.5, ALU.add, ALU.mult)
            for _ in range(4):
                nc.vector.reciprocal(q[:, :], y[:, :])
                nc.vector.tensor_tensor(q[:, :], var[:, :], q[:, :], ALU.mult)
                nc.vector.tensor_tensor(y[:, :], y[:, :], q[:, :], ALU.add)
                nc.vector.tensor_scalar(y[:, :], y[:, :], 0.5, None, ALU.mult)
            nc.vector.reciprocal(rstd[:, :], y[:, :])
            if want_nmr:
                nmr = stp.tile([P, NR], F32, tag="nmr")
                nc.vector.tensor_tensor(nmr[:, :], mv[:, :, 0], rstd[:, :],
                                        ALU.mult)
                nc.vector.tensor_scalar(nmr[:, :], nmr[:, :], -1.0, None,
                                        ALU.mult)
                return mv, rstd, nmr
            return mv, rstd

        def ln_apply(srcs, dst, mv, rstd, gb_tile):
            """dst[:rsz, r, :DIM] = LN of srcs[r] (bf16).  Runs on GPSIMD:
            it sits on the slack-rich prep path and frees DVE cycles."""
            for r in range(NR):
                rsz = RT[r][1]
                nc.gpsimd.tensor_scalar(
                    dst[:rsz, r, :DIM], srcs[r], mv[:rsz, r, 0:1],
                    rstd[:rsz, r:r + 1], ALU.subtract, ALU.mult)
                if gb_tile is not None:
                    nc.gpsimd.tensor_tensor(
                        dst[:rsz, r, :DIM], dst[:rsz, r, :DIM],
                        gb_tile[:rsz, 0, :DIM], ALU.mult)
                    nc.gpsimd.tensor_tensor(
                        dst[:rsz, r, :DIM], dst[:rsz, r, :DIM],
                        gb_tile[:rsz, 1, :DIM], ALU.add)

        def split_hilo(t, hi, lo):
            """hi = fp8(t), lo = fp8(t - hi) per row tile; zero the pad rows
            of the 4th k-plane so DoubleRow k-pair reads see 0, not junk.
            hi + lo on GPSIMD, memsets on GPSIMD (frees DVE for gy-lo)."""
            nc.gpsimd.memset(hi[:, NR - 1, :], 0.0)
            nc.gpsimd.memset(lo[:, NR - 1, :], 0.0)
            for r in range(NR):
                rsz = RT[r][1]
                nc.gpsimd.tensor_copy(hi[:rsz, r, :DIM], t[:rsz, r, :DIM])
                nc.gpsimd.tensor_tensor(lo[:rsz, r, :DIM], t[:rsz, r, :DIM],
                                        hi[:rsz, r, :DIM], ALU.subtract)

        def load_gb(dram_ap, tag):
            t = gbp.tile([P, 2, DIM], F32, tag=tag)
            nc.sync.dma_start(t[:, 0, :], dram_ap[0])
            nc.sync.dma_start(t[:, 1, :], dram_ap[1])
            return t

        def h_rows(s):
            return [h_t[s][:RT[r][1], r, :DIM] for r in range(NR)]

        def xprep(s):
            for r in range(NR):
                r0, rsz = RT[r]
                xs = []
                for c in range(C):
                    xt = xpp.tile([P, WP], F32, tag=f"xp{c}")
                    nc.sync.dma_start(xt[:rsz, :DIM], x_d[s, c, r0:r0 + rsz, :])
                    xs.append(xt)
                hr = h_t[s][:rsz, r, :DIM]
                nc.vector.tensor_tensor(hr, xs[0][:rsz, :DIM], xs[1][:rsz, :DIM], ALU.add)
                nc.vector.tensor_tensor(hr, hr, xs[2][:rsz, :DIM], ALU.add)
                nc.vector.tensor_scalar(hr, hr, 1.0 / C, None, ALU.mult)

        def load_w1pair(hi_ap, lo_ap):
            """[NR, P, TOK] fp8 -> [P, NR, TOK] hi+lo, one DMA per k-plane
            (fewer descriptors -> less HWDGE serial time)."""
            th = wa.tile([P, NR, TOK], FP8, tag="w1h")
            tl = wa.tile([P, NR, TOK], FP8, tag="w1l")
            for k in range(NR):
                nc.sync.dma_start(th[:, k, :], hi_ap[k])
                nc.sync.dma_start(tl[:, k, :], lo_ap[k])
            return th, tl

        def load_w2pair(hi_ap, lo_ap):
            """[NT, P, DIM] fp8 -> [P, NT, W8] hi+lo."""
            th = wb.tile([P, NT, W8], FP8, tag="w2h")
            tl = wb.tile([P, NT, W8], FP8, tag="w2l")
            for k in range(NT):
                nc.sync.dma_start(th[:, k, :DIM], hi_ap[k])
                nc.sync.dma_start(tl[:, k, :DIM], lo_ap[k])
            return th, tl

        def load_b1like(dram_ap):
            t = bp.tile([P, NT], F32, tag="b1")
            nc.sync.dma_start(t[:, :], dram_ap.rearrange("(k p) -> p k", p=P))
            return t

        def load_b2like(dram_ap):
            t = bp.tile([P, NR], F32, tag="b2")
            nc.sync.dma_start(t[:, :3], dram_ap[:384].rearrange("(d p) -> p d", p=P))
            nc.sync.dma_start(t[:43, 3:4], dram_ap[384:, None])
            return t

        def mixer_mms(hh, hl, w1h, w1l, w2h, w2l, b1t, swap2):
            """3-term fp8 DoubleRow chain: per m: MM1 (6 DR passes over 2
            k-pairs x 3 hi/lo terms); gelu -> bf16 pair tile; split pair to
            fp8 hi/lo; MM2 per k-pair j: 3 DR passes x 4 DIM-chunks.
            swap2: MM2 uses the gy pair as STATIONARY and w2 rows MOVING so
            the out lands in h-layout (channel mixer)."""
            accs = [ps_acc.tile([P, WP], F32, tag="acc", name=f"acc{i}") for i in range(NR)]

            def mm2_pair(j, gh, gl):
                first, last = (j == 0), (j == NPAIR - 1)
                for d in range(NR):
                    d0, dsz = RT[d]
                    if swap2:
                        terms = ((gh[:, 0:2, d0:d0 + dsz], w2h[:, 2 * j:2 * j + 2, :DIM]),
                                 (gl[:, 0:2, d0:d0 + dsz], w2h[:, 2 * j:2 * j + 2, :DIM]),
                                 (gh[:, 0:2, d0:d0 + dsz], w2l[:, 2 * j:2 * j + 2, :DIM]))
                    else:
                        terms = ((w2h[:, 2 * j:2 * j + 2, d0:d0 + dsz], gh[:, 0:2, :DIM]),
                                 (w2l[:, 2 * j:2 * j + 2, d0:d0 + dsz], gh[:, 0:2, :DIM]),
                                 (w2h[:, 2 * j:2 * j + 2, d0:d0 + dsz], gl[:, 0:2, :DIM]))
                    for ti, (S, Mv) in enumerate(terms):
                        nc.tensor.matmul(accs[d][:dsz, :DIM], S, Mv,
                                         start=(first and ti == 0),
                                         stop=(last and ti == 2),
                                         perf_mode=DR)

            pend = []          # completed pairs awaiting MM2 (lag 2 pairs)
            gf = gh = gl = None
            for m in range(NT):
                if m % 2 == 0:
                    gf = gyfp.tile([P, 2, WP], BF16, tag="gyf")
                    gh = gyp8.tile([P, 2, W8], FP8, tag="gyh")
                    gl = gyp8.tile([P, 2, W8], FP8, tag="gyl")
                y_ps = ps_mo.tile([P, WP], F32, tag="mo")
                for kp in range(2):
                    k2 = slice(2 * kp, 2 * kp + 2)
                    terms = ((w1h[:, k2, ts(m, P)], hh[:, k2, :DIM]),
                             (w1l[:, k2, ts(m, P)], hh[:, k2, :DIM]),
                             (w1h[:, k2, ts(m, P)], hl[:, k2, :DIM]))
                    for ti, (S, Mv) in enumerate(terms):
                        nc.tensor.matmul(y_ps[:, :DIM], S, Mv,
                                         start=(kp == 0 and ti == 0),
                                         stop=(kp == 1 and ti == 2),
                                         perf_mode=DR)
                if b1t is None:
                    nc.scalar.activation(gf[:, m % 2, :DIM], y_ps[:, :DIM],
                                         AF.Gelu, scale=1.0 / WS)
                else:
                    nc.scalar.activation(gf[:, m % 2, :DIM], y_ps[:, :DIM],
                                         AF.Gelu, scale=1.0 / WS,
                                         bias=b1t[:, m:m + 1])
                if m % 2 == 1:
                    j = m // 2
                    # pair split: hi = fp8(gf) on ACT (one 854-wide copy),
                    # lo = gf - hi on GPSIMD
                    # gy split must stay on low-latency engines: hi on ACT,
                    # lo on DVE (GPSIMD's 0.42-efficiency + shallow queue
                    # stalls the MM2 chain).
                    nc.scalar.activation(gh[:, 0:2, :DIM], gf[:, 0:2, :DIM],
                                         AF.Identity, bias=0.0)
                    nc.vector.tensor_tensor(gl[:, 0:2, :DIM], gf[:, 0:2, :DIM],
                                            gh[:, 0:2, :DIM], ALU.subtract)
                    pend.append((j, gh, gl))
                    if len(pend) > 4:
                        mm2_pair(*pend.pop(0))
            for p_ in pend:
                mm2_pair(*p_)
            return accs

        def transpose_to(dst, src_tile):
            """PE-transpose src_tile (bf16 row-tiles) into dst [P, NR, WP]."""
            for c in range(NR):
                c0, csz = RT[c]
                tp = ps_tp.tile([P, WP], BF16, tag="tp")
                for r in range(NR):
                    r0, rsz = RT[r]
                    nc.tensor.transpose(tp[:csz, r0:r0 + rsz],
                                        src_tile[:rsz, r, c0:c0 + csz],
                                        ident[:rsz, :rsz])
                nc.vector.tensor_copy(dst[:csz, c, :DIM], tp[:csz, :DIM])

        def add_resid(s, accs, b2t, d2bt):
            """h[s] += accs/WS (+ biases); split DVE/GPSIMD for balance."""
            for d in range(NR):
                dsz = RT[d][1]
                hr = h_t[s][:dsz, d, :DIM]
                eng = nc.vector
                eng.scalar_tensor_tensor(hr, accs[d][:dsz, :DIM],
                                         1.0 / WS, hr, ALU.mult, ALU.add)
                if b2t is not None:
                    eng.tensor_scalar_add(hr, hr, b2t[:dsz, d:d + 1])
                if d2bt is not None:
                    eng.tensor_tensor(hr, hr, d2bt[:dsz, :DIM], ALU.add)

        # ---------------- main program ----------------
        depth = int(os.environ.get("KMIX_DEPTH", DEPTH))

        class _Ph:
            pass

        def tok_phase(blk):
            ph = _Ph()

            def load_small():
                ph.b1 = None if b1_triv else load_b1like(rb1_d[blk])
                ph.b2 = None if b2_triv else load_b2like(rb2_d[blk])
                ph.gb = None if ln1_triv else load_gb(ln1g_d[blk], "gb1")

            def load_big():
                ph.w1 = load_w1pair(rw1h_d[blk], rw1l_d[blk])
                ph.w2 = load_w2pair(rw2h_d[blk], rw2l_d[blk])

            def prep(s):
                if blk == 0:
                    xprep(s)
                mn, rstd = ln_stats(h_rows(s))
                t = tsp.tile([P, NR, WP], BF16, tag="t")
                ln_apply(h_rows(s), t, mn, rstd, ph.gb)
                hh = hnp.tile([P, NR, W8], FP8, tag="hnh")
                hl = hnp.tile([P, NR, W8], FP8, tag="hnl")
                split_hilo(t, hh, hl)
                return (hh, hl)

            def mm(s, handle):
                return mixer_mms(handle[0], handle[1], *ph.w1, *ph.w2,
                                 ph.b1, swap2=False)

            def resid(s, accs):
                add_resid(s, accs, ph.b2, None)

            ph.load_small, ph.load_big = load_small, load_big
            ph.prep, ph.mm, ph.resid = prep, mm, resid
            ph.kind = "tok"
            return ph

        def ch_phase(blk):
            ph = _Ph()

            def load_small():
                ph.b1 = None if b1_triv else load_b1like(cb1_d[blk])
                ph.d2 = None
                if not b2_triv:
                    ph.d2 = gbp.tile([P, DIM], F32, tag="d2b")
                    nc.sync.dma_start(ph.d2[:, :], cb2b_d[blk])
                ph.gb = None if ln2_triv else load_gb(ln2g_d[blk], "gb2")

            def load_big():
                ph.w1 = load_w1pair(cw1h_d[blk], cw1l_d[blk])
                ph.w2 = load_w2pair(cw2h_d[blk], cw2l_d[blk])

            def prep(s):
                mn, rstd = ln_stats(h_rows(s))
                t = tsp.tile([P, NR, WP], BF16, tag="t")
                ln_apply(h_rows(s), t, mn, rstd, ph.gb)
                tT = ttp.tile([P, NR, WP], BF16, tag="tT")
                transpose_to(tT, t)
                yh = ytp.tile([P, NR, W8], FP8, tag="yth")
                yl = ytp.tile([P, NR, W8], FP8, tag="ytl")
                split_hilo(tT, yh, yl)
                return (yh, yl)

            def mm(s, handle):
                return mixer_mms(handle[0], handle[1], *ph.w1, *ph.w2,
                                 ph.b1, swap2=True)

            def resid(s, accs):
                add_resid(s, accs, None, ph.d2)

            ph.load_small, ph.load_big = load_small, load_big
            ph.prep, ph.mm, ph.resid = prep, mm, resid
            ph.kind = "ch"
            return ph

        def fin_phase():
            ph = _Ph()

            def load_small():
                load_dmask()
                ph.gbf = None if lnf_triv else load_gb(lnfg_d, "gbf")
                ph.lb = None
                if not lb_triv:
                    ph.lb = gbp.tile([P, DIM], F32, tag="lbb")
                    nc.sync.dma_start(ph.lb[:, :], lb_d)

            def load_big():
                ph.lw = cst.tile([P, NR, WP], BF16, tag="lwf")
                for k in range(NR):
                    k0, ksz = RT[k]
                    nc.sync.dma_start(ph.lw[:ksz, k, :DIM], lw_d[k0:k0 + ksz, :])

            def prep(s):
                if depth == 0:      # debug path: no mixer blocks ran
                    xprep(s)
                mn, rstd = ln_stats(h_rows(s))
                f1 = tsp.tile([P, NR, WP], BF16, tag="t")
                ln_apply(h_rows(s), f1, mn, rstd, ph.gbf)
                f1T = ttp.tile([P, NR, WP], BF16, tag="tT")
                transpose_to(f1T, f1)
                return f1T

            def mm(s, f1T):
                f2 = vbp.tile([P, NR, WP], F32, tag="vb")
                for m in range(NR):
                    m0, msz = RT[m]
                    acc3 = ps_mo.tile([P, WP], F32, tag="mo")
                    for c in range(NR):
                        csz = RT[c][1]
                        nc.tensor.matmul(acc3[:msz, :DIM],
                                         f1T[:csz, c, m0:m0 + msz],
                                         ph.lw[:csz, c, :DIM],
                                         start=(c == 0), stop=(c == NR - 1))
                    nc.scalar.activation(f2[:msz, m, :DIM], acc3[:msz, :DIM],
                                         AF.Identity, bias=0.0)
                    if ph.lb is not None:
                        nc.vector.tensor_tensor(f2[:msz, m, :DIM],
                                                f2[:msz, m, :DIM],
                                                ph.lb[:msz, :DIM], ALU.add)
                f2rows = [f2[:RT[r][1], r, :DIM] for r in range(NR)]
                mn2, rstd2, nmr2 = ln_stats(f2rows, want_nmr=True)
                ot = otp.tile([P, NR, WP], F32, tag="ot")
                for r in range(NR):
                    r0, rsz = RT[r]
                    if ph.gbf is None:
                        nc.scalar.activation(ot[:rsz, r, :DIM], f2rows[r], AF.Abs,
                                             scale=rstd2[:rsz, r:r + 1],
                                             bias=nmr2[:rsz, r:r + 1])
                    else:
                        nc.vector.tensor_scalar(ot[:rsz, r, :DIM], f2rows[r],
                                                mn2[:rsz, r, 0:1],
                                                rstd2[:rsz, r:r + 1],
                                                ALU.subtract, ALU.mult)
                        nc.vector.tensor_tensor(ot[:rsz, r, :DIM],
                                                ot[:rsz, r, :DIM],
                                                ph.gbf[:rsz, 0, :DIM], ALU.mult)
                        nc.vector.tensor_tensor(ot[:rsz, r, :DIM],
                                                ot[:rsz, r, :DIM],
                                                ph.gbf[:rsz, 1, :DIM], ALU.add)
                        nc.scalar.activation(ot[:rsz, r, :DIM],
                                             ot[:rsz, r, :DIM], AF.Abs)
                    nc.vector.tensor_tensor(ot[:rsz, r, :DIM], ot[:rsz, r, :DIM],
                                            dm_t[:rsz, r, :DIM], ALU.mult)
                    nc.sync.dma_start(out_d[s, r0:r0 + rsz, :],
                                      ot[:rsz, r, :DIM])
                return None

            def resid(s, accs):
                pass

            ph.load_small, ph.load_big = load_small, load_big
            ph.prep, ph.mm, ph.resid = prep, mm, resid
            ph.kind = "fin"
            return ph

        phases = []
        for rep in range(reps):
            for blk in range(depth):
                phases.append(tok_phase(blk))
                phases.append(ch_phase(blk))
            phases.append(fin_phase())

        steps = []
        for pi, ph in enumerate(phases):
            if ph.kind == "fin" and pi > 0 and phases[pi - 1].kind == "ch":
                phases[pi - 1].zipf = ph
                continue
            for s in range(SPC):
                steps.append((ph, s))

        handles = {}

        def ensure_loaded(j):
            """Kick off a phase's weight DMAs well before its first step so
            the loads hide under the previous phase's (fast fp8) compute."""
            if j >= len(steps):
                return
            ph = steps[j][0]
            if not getattr(ph, "loaded", False):
                ph.load_small()
                ph.load_big()
                ph.loaded = True

        def emit_prep(j):
            if j >= len(steps):
                return
            ph, s = steps[j]
            handles[j] = ph.prep(s)

        # Warmup: sample-0's x DMAs must beat the bulk weight prefetch into
        # the (FIFO) DMA queue: x loads, phase-0 weights, then the lookahead
        # prefetch of later phases.
        ph0 = steps[0][0]
        ph0.load_small()
        emit_prep(0)
        ph0.load_big()
        ph0.loaded = True
        emit_prep(1)
        emit_prep(2)
        emit_prep(3)
        for jj in range(1, 7):
            ensure_loaded(jj)
        pend = None        # (resid_fn, s, accs) awaiting residual add
        for j, (ph, s) in enumerate(steps):
            ensure_loaded(j + 6)
            if j + 2 not in handles and j + 2 < len(steps):
                emit_prep(j + 2)
            zipf = getattr(ph, "zipf", None)
            fh = None
            if zipf is not None and s >= 0:
                if not getattr(zipf, "loaded", False):
                    zipf.load_small()
                    zipf.load_big()
                    zipf.loaded = True
                if s >= 2:
                    fh = zipf.prep(s - 2)
            if pend is not None:
                pend[0](pend[1], pend[2])
                pend = None
            accs = ph.mm(s, handles.pop(j))
            if accs is not None:
                pend = (ph.resid, s, accs)
            if fh is not None:
                zipf.mm(s - 2, fh)
            if zipf is not None and s == SPC - 1:
                if pend is not None:
                    pend[0](pend[1], pend[2])
                    pend = None
                fhs = [zipf.prep(s2) for s2 in (SPC - 2, SPC - 1)]
                for s2, fh2 in zip((SPC - 2, SPC - 1), fhs):
                    zipf.mm(s2, fh2)
        if pend is not None:
            pend[0](pend[1], pend[2])

    nc.compile()
    return nc


def _host_prep(inputs):
    g = {k: np.asarray(v, dtype=np.float32) for k, v in inputs.items()}
    ln1_triv = bool(np.all(g["ln1_g"] == 1.0) and np.all(g["ln1_b"] == 0.0))
    ln2_triv = bool(np.all(g["ln2_g"] == 1.0) and np.all(g["ln2_b"] == 0.0))
    lnf_triv = bool(np.all(g["lnf_g"] == 1.0) and np.all(g["lnf_b"] == 0.0))
    lb_triv = bool(np.all(g["lb"] == 0.0))
    b1_triv = bool(np.all(g["rb1"] == 0.0) and np.all(g["cb1"] == 0.0))
    b2_triv = bool(np.all(g["rb2"] == 0.0) and np.all(g["cb2"] == 0.0))
    flags = (ln1_triv, ln2_triv, lnf_triv, lb_triv, b1_triv, b2_triv)

    dmask = np.ones((NR, P, DIM), dtype=np.float32)
    for r in range(NR):
        for p in range(min(P, DIM - 128 * r)):
            dmask[r, p, 128 * r + p] = 0.0

    def bcast2(gv, bv):
        gb = np.stack([gv, bv], axis=-2)[..., None, :]
        return np.broadcast_to(gb, gb.shape[:-2] + (P, DIM)).copy()

    bf = ml_dtypes.bfloat16
    f8 = ml_dtypes.float8_e4m3

    def split_w1(w):  # [DEPTH, DIM, TOK] -> hi/lo [DEPTH, NR, P, TOK]
        wp = np.zeros((DEPTH, NR * P, TOK), np.float32)
        wp[:, :DIM] = w * WS
        hi = wp.astype(f8)
        lo = (wp - hi.astype(np.float32)).astype(f8)
        return (np.ascontiguousarray(hi.reshape(DEPTH, NR, P, TOK)),
                np.ascontiguousarray(lo.reshape(DEPTH, NR, P, TOK)))

    def split_w2(w):  # [DEPTH, TOK, DIM] -> hi/lo [DEPTH, NT, P, DIM]
        ws = w * WS
        hi = ws.astype(f8)
        lo = (ws - hi.astype(np.float32)).astype(f8)
        return (np.ascontiguousarray(hi.reshape(DEPTH, NT, P, DIM)),
                np.ascontiguousarray(lo.reshape(DEPTH, NT, P, DIM)))

    rw1h, rw1l = split_w1(g["rw1"])
    rw2h, rw2l = split_w2(g["rw2"])
    cw1h, cw1l = split_w1(g["cw1"])
    cw2h, cw2l = split_w2(g["cw2"])
    common = {
        "rw1h": rw1h, "rw1l": rw1l, "rw2h": rw2h, "rw2l": rw2l,
        "cw1h": cw1h, "cw1l": cw1l, "cw2h": cw2h, "cw2l": cw2l,
        "lw": np.ascontiguousarray(g["lw"].astype(bf)),
        "dmask": dmask,
    }
    if not b1_triv:
        common["rb1"] = g["rb1"]
        common["cb1"] = g["cb1"]
    if not b2_triv:
        common["rb2"] = g["rb2"]
        common["cb2b"] = np.ascontiguousarray(
            np.broadcast_to(g["cb2"][:, None, :], (DEPTH, P, DIM)))
    if not ln1_triv:
        common["ln1gb"] = np.ascontiguousarray(bcast2(g["ln1_g"], g["ln1_b"]))
    if not ln2_triv:
        common["ln2gb"] = np.ascontiguousarray(bcast2(g["ln2_g"], g["ln2_b"]))
    if not lnf_triv:
        common["lnfgb"] = np.ascontiguousarray(bcast2(g["lnf_g"], g["lnf_b"]))
    if not lb_triv:
        common["lbb"] = np.broadcast_to(g["lb"][None, :], (P, DIM)).copy()
    x = np.ascontiguousarray(g["x"])
    in_maps = [dict(common, x=np.ascontiguousarray(x[c * SPC:(c + 1) * SPC]))
               for c in range(NCORES)]
    return flags, in_maps


def _get_nc(flags, reps=1):
    key = flags + (reps,)
    if key not in _BUILD_CACHE:
        _BUILD_CACHE[key] = _build(*flags, reps=reps)
    return _BUILD_CACHE[key]


def kernel(**inputs):
    flags, in_maps = _host_prep(inputs)
    nc = _get_nc(flags)
    res = run_bass_kernel_spmd(nc, in_maps, list(range(NCORES)))
    return np.concatenate([res.results[c]["out"] for c in range(NCORES)], axis=0)


def measure_hw_time(inputs, r_hi=4, iters=5):
    """Wall-clock repetition-slope timing on the real device."""
    import time as _time
    flags, in_maps = _host_prep(inputs)
    cores = list(range(NCORES))
    t = {}
    for r in (1, r_hi):
        nc = _get_nc(flags, reps=r)
        best = float("inf")
        run_bass_kernel_spmd(nc, in_maps, cores)  # warm (jit + neff cache)
        for _ in range(iters):
            t0 = _time.perf_counter()
            run_bass_kernel_spmd(nc, in_maps, cores)
            best = min(best, _time.perf_counter() - t0)
        t[r] = best
        print(f"  reps={r}: best wall {best*1e3:.1f} ms")
    return (t[r_hi] - t[1]) / (r_hi - 1) * 1e9


def kernel_traced(**inputs):
    """Like kernel() but with NTFF tracing when available."""
    flags, in_maps = _host_prep(inputs)
    nc = _get_nc(flags)
    try:
        res = run_bass_kernel_spmd(nc, in_maps, list(range(NCORES)), trace=True)
    except ModuleNotFoundError:
        res = run_bass_kernel_spmd(nc, in_maps, list(range(NCORES)))
    out = np.concatenate([res.results[c]["out"] for c in range(NCORES)], axis=0)
    return out, res


# revision 37
# speedup vs baseline: 1.2409x; 1.0033x over previous
"""MLP-Mixer forward on 8 Trainium2 NeuronCores, data-parallel over batch.

Strategy
--------
Pure data parallel: 64 samples -> 8 per core; all parameters replicated.
Per core, the 8 residual states h[s] (427x427 fp32) stay resident in SBUF
across all 8 mixer blocks; block weights stream from HBM into
double-buffered SBUF slots.

All mixer matmuls run as fp8-e4m3 DoubleRow ("3-term hi/lo split"):
every operand X is split into X_hi = fp8(X) and X_lo = fp8(X - X_hi);
the product uses Wh@Ah + Wl@Ah + Wh@Al (the lo*lo term is dropped,
~1e-4 relative).  Each DoubleRow pass contracts a k-PAIR (2x128 rows)
at 0.5 cyc/row, so a logical K-tile costs 1.5 passes x 0.5 = 0.75x the
bf16 rate while keeping bf16-level accuracy (verified 4.4e-3 vs the
2e-2 budget in an exact numpy emulation; every single-fp8 operand
variant measured >3e-2 and was rejected).

Weights are host-split (hi/lo at scale 64, K padded 427->512 with zeros
so the garbage rows of the 4th activation k-plane are annihilated; the
LN-scratch pad rows are memset on GPSIMD so 0*junk can't make NaN).
Activations split on-chip, with engine choice driven by queue-latency
measurements in TimelineSim (the graded metric):
  - hn/ytT: DVE writes the LN to a bf16 scratch t via GPSIMD tensor_scalar
    (prep path, slack-rich), GPSIMD copies t->hi(fp8), DVE t-hi->lo(fp8).
  - gy: ACT gelu (descale 1/64 folded into the activation scale) writes a
    bf16 pair tile; hi = ACT Identity pair-copy, lo = DVE pair-subtract.
    GPSIMD is NOT usable here: 0.42 sw-efficiency + its shallow queue
    stalls the MM2 chain (measured +0.4ms).
  - residual h += acc/64 via DVE scalar_tensor_tensor.

Layouts per sample (row tiles = 4 chunks of 128|43 partitions):
  h    [128, 4, 428] fp32      residual stream
  t    [128, 4, 428] bf16      LN scratch (also the final-stage f1)
  hn_h/hn_l [128, 4, 432] fp8  split LN1(h), k-planes for MM1 pairs
  gy pairs  [128, 2, 432] fp8  split gelu out, planes = MM2 k-pairs
  token: yT = w1.T @ hn (m=TOK tiles); zT = w2.T @ gy -> h-layout
  channel: t -> PE-transpose(bf16) -> split -> uT = c1.T @ ytT;
           MM2 swapped (gy stationary, c2 moving) -> h-layout
  final stage: all-bf16 (f1T transpose + lw matmul + LNf/abs/mask);
  fp8 would cost ~4e-2 error here (no residual dilution), bf16 is
  only ~3% of the matmul budget.

LayerNorm stats: single-pass DVE bn_stats/bn_aggr + Newton sqrt (keeps
ScalarE on the gelu table — no table containing gelu also has rsqrt).
Scheduling: the baseline software pipeline is preserved (prep two
samples ahead, final stage zipped into the last channel phase), with:
MM2 lagging MM1 by 4 k-pairs (hides the gelu->hi->lo split chain),
3 MM1-psum bufs (gelu on the ~80%-busy ACT released the 2-buf pool too
late; +108us), phase weights prefetched a full phase early as one DMA
per k-plane (HWDGE descriptor-gen serial time 725->185us), and the
warmup ordered x0,prep0,x1-weights so sample-0 compute starts ASAP.
TimelineSim: 2.481 ms/core (PE 91.6% busy; 3-term fp8 matmul floor
2.19ms, bf16 baseline was 3.079 ms).
"""

import os
from contextlib import ExitStack

import numpy as np
import ml_dtypes

import concourse.bacc as bacc
import concourse.tile as tile
import concourse.mybir as mybir
from concourse.bass import ts
from concourse.bass_utils import run_bass_kernel_spmd
from concourse.masks import make_identity

B, C, DIM, DEPTH, TOK = 64, 3, 427, 8, 2048
NCORES = 8
SPC = B // NCORES           # samples per core
P = 128
WP = DIM + 1                # 428, bf16/f32 tile stride
W8 = 432                    # fp8 tile stride (16-byte aligned)
NT = TOK // P               # 16
NPAIR = NT // 2             # 8 k-pairs for the second matmul
RT = [(0, 128), (128, 128), (256, 128), (384, 43)]  # DIM row/k tiles
NR = len(RT)
EPS = 1e-5
WS = 64.0                   # weight scale before fp8 quantization
F32 = mybir.dt.float32
BF16 = mybir.dt.bfloat16
FP8 = mybir.dt.float8e4
AF = mybir.ActivationFunctionType
ALU = mybir.AluOpType
DR = mybir.MatmulPerfMode.DoubleRow

_BUILD_CACHE = {}


def _build(ln1_triv, ln2_triv, lnf_triv, lb_triv, b1_triv, b2_triv,
           reps=1):
    nc = bacc.Bacc("TRN2", target_bir_lowering=False, debug=False,
                   num_devices=NCORES)

    x_d = nc.dram_tensor("x", [SPC, C, DIM, DIM], F32, kind="ExternalInput").ap()
    # fp8 hi/lo weight pairs; first-matmul weights K-padded 427->512 (zeros)
    rw1h_d = nc.dram_tensor("rw1h", [DEPTH, NR, P, TOK], FP8, kind="ExternalInput").ap()
    rw1l_d = nc.dram_tensor("rw1l", [DEPTH, NR, P, TOK], FP8, kind="ExternalInput").ap()
    rw2h_d = nc.dram_tensor("rw2h", [DEPTH, NT, P, DIM], FP8, kind="ExternalInput").ap()
    rw2l_d = nc.dram_tensor("rw2l", [DEPTH, NT, P, DIM], FP8, kind="ExternalInput").ap()
    cw1h_d = nc.dram_tensor("cw1h", [DEPTH, NR, P, TOK], FP8, kind="ExternalInput").ap()
    cw1l_d = nc.dram_tensor("cw1l", [DEPTH, NR, P, TOK], FP8, kind="ExternalInput").ap()
    cw2h_d = nc.dram_tensor("cw2h", [DEPTH, NT, P, DIM], FP8, kind="ExternalInput").ap()
    cw2l_d = nc.dram_tensor("cw2l", [DEPTH, NT, P, DIM], FP8, kind="ExternalInput").ap()
    lw_d = nc.dram_tensor("lw", [DIM, DIM], BF16, kind="ExternalInput").ap()
    dmask_d = nc.dram_tensor("dmask", [NR, P, DIM], F32, kind="ExternalInput").ap()
    rb1_d = rb2_d = cb1_d = cb2b_d = None
    if not b1_triv:
        rb1_d = nc.dram_tensor("rb1", [DEPTH, TOK], F32, kind="ExternalInput").ap()
        cb1_d = nc.dram_tensor("cb1", [DEPTH, TOK], F32, kind="ExternalInput").ap()
    if not b2_triv:
        rb2_d = nc.dram_tensor("rb2", [DEPTH, DIM], F32, kind="ExternalInput").ap()
        cb2b_d = nc.dram_tensor("cb2b", [DEPTH, P, DIM], F32, kind="ExternalInput").ap()
    ln1g_d = ln2g_d = lnfg_d = lb_d = None
    if not ln1_triv:
        ln1g_d = nc.dram_tensor("ln1gb", [DEPTH, 2, P, DIM], F32, kind="ExternalInput").ap()
    if not ln2_triv:
        ln2g_d = nc.dram_tensor("ln2gb", [DEPTH, 2, P, DIM], F32, kind="ExternalInput").ap()
    if not lnf_triv:
        lnfg_d = nc.dram_tensor("lnfgb", [2, P, DIM], F32, kind="ExternalInput").ap()
    if not lb_triv:
        lb_d = nc.dram_tensor("lbb", [P, DIM], F32, kind="ExternalInput").ap()
    out_d = nc.dram_tensor("out", [SPC, DIM, DIM], F32, kind="ExternalOutput").ap()

    with tile.TileContext(nc) as tc, ExitStack() as ctx:
        hp = ctx.enter_context(tc.tile_pool(name="h", bufs=SPC))
        wa = ctx.enter_context(tc.tile_pool(name="wa", bufs=2))
        wb = ctx.enter_context(tc.tile_pool(name="wb", bufs=2))
        tsp = ctx.enter_context(tc.tile_pool(name="tscr", bufs=4))
        ttp = ctx.enter_context(tc.tile_pool(name="tT", bufs=4))
        hnp = ctx.enter_context(tc.tile_pool(name="hn8", bufs=3))
        ytp = ctx.enter_context(tc.tile_pool(name="yt8", bufs=3))
        gyfp = ctx.enter_context(tc.tile_pool(name="gyf", bufs=3))
        gyp8 = ctx.enter_context(tc.tile_pool(name="gy8", bufs=6))
        vbp = ctx.enter_context(tc.tile_pool(name="vb", bufs=1))
        otp = ctx.enter_context(tc.tile_pool(name="ot", bufs=1))
        sqp = ctx.enter_context(tc.tile_pool(name="sq", bufs=1))
        xpp = ctx.enter_context(tc.tile_pool(name="xp", bufs=1))
        cst = ctx.enter_context(tc.tile_pool(name="cst", bufs=1))
        bp = ctx.enter_context(tc.tile_pool(name="bias", bufs=2))
        stp = ctx.enter_context(tc.tile_pool(name="st", bufs=2))
        gbp = ctx.enter_context(tc.tile_pool(name="gb", bufs=2))
        ps_mo = ctx.enter_context(tc.tile_pool(name="mo", bufs=3, space="PSUM"))
        ps_acc = ctx.enter_context(tc.tile_pool(name="acc", bufs=4, space="PSUM"))
        ps_tp = ctx.enter_context(tc.tile_pool(name="tp", bufs=1, space="PSUM"))

        # persistent tiles
        h_t = [hp.tile([P, NR, WP], F32, tag="h", name=f"h{i}") for i in range(SPC)]
        ident = cst.tile([P, P], BF16, tag="ident")
        make_identity(nc, ident[:, :])
        dm_t = cst.tile([P, NR, DIM], F32, tag="dmask")
        dm_loaded = [False]

        def load_dmask():
            if not dm_loaded[0]:
                dm_loaded[0] = True
                for r in range(NR):
                    nc.sync.dma_start(dm_t[:, r, :], dmask_d[r])

        def ln_stats(srcs, want_nmr=False):
            """srcs: list of NR APs [rsz, DIM]. Returns (mv, rstd[, nmr])
            as [P, NR] tiles (column r = row-tile r)."""
            st6 = stp.tile([P, NR, 6], F32, tag="st6")
            mv = stp.tile([P, NR, 2], F32, tag="mv")
            for r, src in enumerate(srcs):
                rsz = RT[r][1]
                nc.vector.bn_stats(st6[:rsz, r, :], src)
                nc.vector.bn_aggr(mv[:rsz, r, :], st6[:rsz, r, :])
            var = stp.tile([P, NR], F32, tag="var")
            y = stp.tile([P, NR], F32, tag="nwy")
            q = stp.tile([P, NR], F32, tag="nwq")
            rstd = stp.tile([P, NR], F32, tag="rstd")
            nc.vector.tensor_scalar(var[:, :], mv[:, :, 1], EPS, None, ALU.add)
            # Newton sqrt: y0 = 0.5*(1+v); y <- 0.5*(y + v/y)  (4 iters)
            nc.vector.tensor_scalar(y[:, :], var[:, :], 1.0, 0.5, ALU.add, ALU.mult)
            for _ in range(4):
                nc.vector.reciprocal(q[:, :], y[:, :])
                nc.vector.tensor_tensor(q[:, :], var[:, :], q[:, :], ALU.mult)
                nc.vector.tensor_tensor(y[:, :], y[:, :], q[:, :], ALU.add)
                nc.vector.tensor_scalar(y[:, :], y[:, :], 0.5, None, ALU.mult)
            nc.vector.reciprocal(rstd[:, :], y[:, :])
            if want_nmr:
                nmr = stp.tile([P, NR], F32, tag="nmr")
                nc.vector.tensor_tensor(nmr[:, :], mv[:, :, 0], rstd[:, :],
                                        ALU.mult)
                nc.vector.tensor_scalar(nmr[:, :], nmr[:, :], -1.0, None,
                                        ALU.mult)
                return mv, rstd, nmr
            return mv, rstd

        def ln_apply(srcs, dst, mv, rstd, gb_tile):
            """dst[:rsz, r, :DIM] = LN of srcs[r] (bf16).  Runs on GPSIMD:
            it sits on the slack-rich prep path and frees DVE cycles."""
            for r in range(NR):
                rsz = RT[r][1]
                nc.gpsimd.tensor_scalar(
                    dst[:rsz, r, :DIM], srcs[r], mv[:rsz, r, 0:1],
                    rstd[:rsz, r:r + 1], ALU.subtract, ALU.mult)
                if gb_tile is not None:
                    nc.gpsimd.tensor_tensor(
                        dst[:rsz, r, :DIM], dst[:rsz, r, :DIM],
                        gb_tile[:rsz, 0, :DIM], ALU.mult)
                    nc.gpsimd.tensor_tensor(
                        dst[:rsz, r, :DIM], dst[:rsz, r, :DIM],
                        gb_tile[:rsz, 1, :DIM], ALU.add)

        def split_hilo(t, hi, lo):
            """hi = fp8(t), lo = fp8(t - hi) per row tile; zero the pad rows
            of the 4th k-plane so DoubleRow k-pair reads see 0, not junk.
            hi + lo on GPSIMD, memsets on GPSIMD (frees DVE for gy-lo)."""
            nc.gpsimd.memset(hi[:, NR - 1, :], 0.0)
            nc.gpsimd.memset(lo[:, NR - 1, :], 0.0)
            for r in range(NR):
                rsz = RT[r][1]
                nc.gpsimd.tensor_copy(hi[:rsz, r, :DIM], t[:rsz, r, :DIM])
                nc.gpsimd.tensor_tensor(lo[:rsz, r, :DIM], t[:rsz, r, :DIM],
                                        hi[:rsz, r, :DIM], ALU.subtract)

        def load_gb(dram_ap, tag):
            t = gbp.tile([P, 2, DIM], F32, tag=tag)
            nc.sync.dma_start(t[:, 0, :], dram_ap[0])
            nc.sync.dma_start(t[:, 1, :], dram_ap[1])
            return t

        def h_rows(s):
            return [h_t[s][:RT[r][1], r, :DIM] for r in range(NR)]

        def xprep(s):
            for r in range(NR):
                r0, rsz = RT[r]
                xs = []
                for c in range(C):
                    xt = xpp.tile([P, WP], F32, tag=f"xp{c}")
                    nc.sync.dma_start(xt[:rsz, :DIM], x_d[s, c, r0:r0 + rsz, :])
                    xs.append(xt)
                hr = h_t[s][:rsz, r, :DIM]
                nc.vector.tensor_tensor(hr, xs[0][:rsz, :DIM], xs[1][:rsz, :DIM], ALU.add)
                nc.vector.tensor_tensor(hr, hr, xs[2][:rsz, :DIM], ALU.add)
                nc.vector.tensor_scalar(hr, hr, 1.0 / C, None, ALU.mult)

        def load_w1pair(hi_ap, lo_ap):
            """[NR, P, TOK] fp8 -> [P, NR, TOK] hi+lo, one DMA per k-plane
            (fewer descriptors -> less HWDGE serial time)."""
            th = wa.tile([P, NR, TOK], FP8, tag="w1h")
            tl = wa.tile([P, NR, TOK], FP8, tag="w1l")
            for k in range(NR):
                nc.sync.dma_start(th[:, k, :], hi_ap[k])
                nc.sync.dma_start(tl[:, k, :], lo_ap[k])
            return th, tl

        def load_w2pair(hi_ap, lo_ap):
            """[NT, P, DIM] fp8 -> [P, NT, W8] hi+lo."""
            th = wb.tile([P, NT, W8], FP8, tag="w2h")
            tl = wb.tile([P, NT, W8], FP8, tag="w2l")
            for k in range(NT):
                nc.sync.dma_start(th[:, k, :DIM], hi_ap[k])
                nc.sync.dma_start(tl[:, k, :DIM], lo_ap[k])
            return th, tl

        def load_b1like(dram_ap):
            t = bp.tile([P, NT], F32, tag="b1")
            nc.sync.dma_start(t[:, :], dram_ap.rearrange("(k p) -> p k", p=P))
            return t

        def load_b2like(dram_ap):
            t = bp.tile([P, NR], F32, tag="b2")
            nc.sync.dma_start(t[:, :3], dram_ap[:384].rearrange("(d p) -> p d", p=P))
            nc.sync.dma_start(t[:43, 3:4], dram_ap[384:, None])
            return t

        def mixer_mms(hh, hl, w1h, w1l, w2h, w2l, b1t, swap2):
            """3-term fp8 DoubleRow chain: per m: MM1 (6 DR passes over 2
            k-pairs x 3 hi/lo terms); gelu -> bf16 pair tile; split pair to
            fp8 hi/lo; MM2 per k-pair j: 3 DR passes x 4 DIM-chunks.
            swap2: MM2 uses the gy pair as STATIONARY and w2 rows MOVING so
            the out lands in h-layout (channel mixer)."""
            accs = [ps_acc.tile([P, WP], F32, tag="acc", name=f"acc{i}") for i in range(NR)]

            def mm2_pair(j, gh, gl):
                first, last = (j == 0), (j == NPAIR - 1)
                for d in range(NR):
                    d0, dsz = RT[d]
                    if swap2:
                        terms = ((gh[:, 0:2, d0:d0 + dsz], w2h[:, 2 * j:2 * j + 2, :DIM]),
                                 (gl[:, 0:2, d0:d0 + dsz], w2h[:, 2 * j:2 * j + 2, :DIM]),
                                 (gh[:, 0:2, d0:d0 + dsz], w2l[:, 2 * j:2 * j + 2, :DIM]))
                    else:
                        terms = ((w2h[:, 2 * j:2 * j + 2, d0:d0 + dsz], gh[:, 0:2, :DIM]),
                                 (w2l[:, 2 * j:2 * j + 2, d0:d0 + dsz], gh[:, 0:2, :DIM]),
                                 (w2h[:, 2 * j:2 * j + 2, d0:d0 + dsz], gl[:, 0:2, :DIM]))
                    for ti, (S, Mv) in enumerate(terms):
                        nc.tensor.matmul(accs[d][:dsz, :DIM], S, Mv,
                                         start=(first and ti == 0),
                                         stop=(last and ti == 2),
                                         perf_mode=DR)

            pend = []          # completed pairs awaiting MM2 (lag 2 pairs)
            gf = gh = gl = None
            for m in range(NT):
                if m % 2 == 0:
                    gf = gyfp.tile([P, 2, WP], BF16, tag="gyf")
                    gh = gyp8.tile([P, 2, W8], FP8, tag="gyh")
                    gl = gyp8.tile([P, 2, W8], FP8, tag="gyl")
                y_ps = ps_mo.tile([P, WP], F32, tag="mo")
                for kp in range(2):
                    k2 = slice(2 * kp, 2 * kp + 2)
                    terms = ((w1h[:, k2, ts(m, P)], hh[:, k2, :DIM]),
                             (w1l[:, k2, ts(m, P)], hh[:, k2, :DIM]),
                             (w1h[:, k2, ts(m, P)], hl[:, k2, :DIM]))
                    for ti, (S, Mv) in enumerate(terms):
                        nc.tensor.matmul(y_ps[:, :DIM], S, Mv,
                                         start=(kp == 0 and ti == 0),
                                         stop=(kp == 1 and ti == 2),
                                         perf_mode=DR)
                if b1t is None:
                    nc.scalar.activation(gf[:, m % 2, :DIM], y_ps[:, :DIM],
                                         AF.Gelu, scale=1.0 / WS)
                else:
                    nc.scalar.activation(gf[:, m % 2, :DIM], y_ps[:, :DIM],
                                         AF.Gelu, scale=1.0 / WS,
                                         bias=b1t[:, m:m + 1])
                if m % 2 == 1:
                    j = m // 2
                    # pair split: hi = fp8(gf) on ACT (one 854-wide copy),
                    # lo = gf - hi on GPSIMD
                    # gy split must stay on low-latency engines: hi on ACT,
                    # lo on DVE (GPSIMD's 0.42-efficiency + shallow queue
                    # stalls the MM2 chain).
                    nc.scalar.activation(gh[:, 0:2, :DIM], gf[:, 0:2, :DIM],
                                         AF.Identity, bias=0.0)
                    nc.vector.tensor_tensor(gl[:, 0:2, :DIM], gf[:, 0:2, :DIM],
                                            gh[:, 0:2, :DIM], ALU.subtract)
                    pend.append((j, gh, gl))
                    if len(pend) > 4:
                        mm2_pair(*pend.pop(0))
            for p_ in pend:
                mm2_pair(*p_)
            return accs

        def transpose_to(dst, src_tile):
            """PE-transpose src_tile (bf16 row-tiles) into dst [P, NR, WP]."""
            for c in range(NR):
                c0, csz = RT[c]
                tp = ps_tp.tile([P, WP], BF16, tag="tp")
                for r in range(NR):
                    r0, rsz = RT[r]
                    nc.tensor.transpose(tp[:csz, r0:r0 + rsz],
                                        src_tile[:rsz, r, c0:c0 + csz],
                                        ident[:rsz, :rsz])
                nc.vector.tensor_copy(dst[:csz, c, :DIM], tp[:csz, :DIM])

        def add_resid(s, accs, b2t, d2bt):
            """h[s] += accs/WS (+ biases); split DVE/GPSIMD for balance."""
            for d in range(NR):
                dsz = RT[d][1]
                hr = h_t[s][:dsz, d, :DIM]
                eng = nc.vector
                eng.scalar_tensor_tensor(hr, accs[d][:dsz, :DIM],
                                         1.0 / WS, hr, ALU.mult, ALU.add)
                if b2t is not None:
                    eng.tensor_scalar_add(hr, hr, b2t[:dsz, d:d + 1])
                if d2bt is not None:
                    eng.tensor_tensor(hr, hr, d2bt[:dsz, :DIM], ALU.add)

        # ---------------- main program ----------------
        depth = int(os.environ.get("KMIX_DEPTH", DEPTH))

        class _Ph:
            pass

        def tok_phase(blk):
            ph = _Ph()

            def load_small():
                ph.b1 = None if b1_triv else load_b1like(rb1_d[blk])
                ph.b2 = None if b2_triv else load_b2like(rb2_d[blk])
                ph.gb = None if ln1_triv else load_gb(ln1g_d[blk], "gb1")

            def load_big():
                ph.w1 = load_w1pair(rw1h_d[blk], rw1l_d[blk])
                ph.w2 = load_w2pair(rw2h_d[blk], rw2l_d[blk])

            def prep(s):
                if blk == 0:
                    xprep(s)
                mn, rstd = ln_stats(h_rows(s))
                t = tsp.tile([P, NR, WP], BF16, tag="t")
                ln_apply(h_rows(s), t, mn, rstd, ph.gb)
                hh = hnp.tile([P, NR, W8], FP8, tag="hnh")
                hl = hnp.tile([P, NR, W8], FP8, tag="hnl")
                split_hilo(t, hh, hl)
                return (hh, hl)

            def mm(s, handle):
                return mixer_mms(handle[0], handle[1], *ph.w1, *ph.w2,
                                 ph.b1, swap2=False)

            def resid(s, accs):
                add_resid(s, accs, ph.b2, None)

            ph.load_small, ph.load_big = load_small, load_big
            ph.prep, ph.mm, ph.resid = prep, mm, resid
            ph.kind = "tok"
            return ph

        def ch_phase(blk):
            ph = _Ph()

            def load_small():
                ph.b1 = None if b1_triv else load_b1like(cb1_d[blk])
                ph.d2 = None
                if not b2_triv:
                    ph.d2 = gbp.tile([P, DIM], F32, tag="d2b")
                    nc.sync.dma_start(ph.d2[:, :], cb2b_d[blk])
                ph.gb = None if ln2_triv else load_gb(ln2g_d[blk], "gb2")

            def load_big():
                ph.w1 = load_w1pair(cw1h_d[blk], cw1l_d[blk])
                ph.w2 = load_w2pair(cw2h_d[blk], cw2l_d[blk])

            def prep(s):
                mn, rstd = ln_stats(h_rows(s))
                t = tsp.tile([P, NR, WP], BF16, tag="t")
                ln_apply(h_rows(s), t, mn, rstd, ph.gb)
                tT = ttp.tile([P, NR, WP], BF16, tag="tT")
                transpose_to(tT, t)
                yh = ytp.tile([P, NR, W8], FP8, tag="yth")
                yl = ytp.tile([P, NR, W8], FP8, tag="ytl")
                split_hilo(tT, yh, yl)
                return (yh, yl)

            def mm(s, handle):
                return mixer_mms(handle[0], handle[1], *ph.w1, *ph.w2,
                                 ph.b1, swap2=True)

            def resid(s, accs):
                add_resid(s, accs, None, ph.d2)

            ph.load_small, ph.load_big = load_small, load_big
            ph.prep, ph.mm, ph.resid = prep, mm, resid
            ph.kind = "ch"
            return ph

        def fin_phase():
            ph = _Ph()

            def load_small():
                load_dmask()
                ph.gbf = None if lnf_triv else load_gb(lnfg_d, "gbf")
                ph.lb = None
                if not lb_triv:
                    ph.lb = gbp.tile([P, DIM], F32, tag="lbb")
                    nc.sync.dma_start(ph.lb[:, :], lb_d)

            def load_big():
                ph.lw = cst.tile([P, NR, WP], BF16, tag="lwf")
                for k in range(NR):
                    k0, ksz = RT[k]
                    nc.sync.dma_start(ph.lw[:ksz, k, :DIM], lw_d[k0:k0 + ksz, :])

            def prep(s):
                if depth == 0:      # debug path: no mixer blocks ran
                    xprep(s)
                mn, rstd = ln_stats(h_rows(s))
                f1 = tsp.tile([P, NR, WP], BF16, tag="t")
                ln_apply(h_rows(s), f1, mn, rstd, ph.gbf)
                f1T = ttp.tile([P, NR, WP], BF16, tag="tT")
                transpose_to(f1T, f1)
                return f1T

            def mm(s, f1T):
                f2 = vbp.tile([P, NR, WP], F32, tag="vb")
                for m in range(NR):
                    m0, msz = RT[m]
                    acc3 = ps_mo.tile([P, WP], F32, tag="mo")
                    for c in range(NR):
                        csz = RT[c][1]
                        nc.tensor.matmul(acc3[:msz, :DIM],
                                         f1T[:csz, c, m0:m0 + msz],
                                         ph.lw[:csz, c, :DIM],
                                         start=(c == 0), stop=(c == NR - 1))
                    nc.scalar.activation(f2[:msz, m, :DIM], acc3[:msz, :DIM],
                                         AF.Identity, bias=0.0)
                    if ph.lb is not None:
                        nc.vector.tensor_tensor(f2[:msz, m, :DIM],
                                                f2[:msz, m, :DIM],
                                                ph.lb[:msz, :DIM], ALU.add)
                f2rows = [f2[:RT[r][1], r, :DIM] for r in range(NR)]
                mn2, rstd2, nmr2 = ln_stats(f2rows, want_nmr=True)
                ot = otp.tile([P, NR, WP], F32, tag="ot")
                for r in range(NR):
                    r0, rsz = RT[r]
                    if ph.gbf is None:
                        nc.scalar.activation(ot[:rsz, r, :DIM], f2rows[r], AF.Abs,
                                             scale=rstd2[:rsz, r:r + 1],
                                             bias=nmr2[:rsz, r:r + 1])
                    else:
                        nc.vector.tensor_scalar(ot[:rsz, r, :DIM], f2rows[r],
                                                mn2[:rsz, r, 0:1],
                                                rstd2[:rsz, r:r + 1],
                                                ALU.subtract, ALU.mult)
                        nc.vector.tensor_tensor(ot[:rsz, r, :DIM],
                                                ot[:rsz, r, :DIM],
                                                ph.gbf[:rsz, 0, :DIM], ALU.mult)
                        nc.vector.tensor_tensor(ot[:rsz, r, :DIM],
                                                ot[:rsz, r, :DIM],
                                                ph.gbf[:rsz, 1, :DIM], ALU.add)
                        nc.scalar.activation(ot[:rsz, r, :DIM],
                                             ot[:rsz, r, :DIM], AF.Abs)
                    nc.vector.tensor_tensor(ot[:rsz, r, :DIM], ot[:rsz, r, :DIM],
                                            dm_t[:rsz, r, :DIM], ALU.mult)
                    nc.sync.dma_start(out_d[s, r0:r0 + rsz, :],
                                      ot[:rsz, r, :DIM])
                return None

            def resid(s, accs):
                pass

            ph.load_small, ph.load_big = load_small, load_big
            ph.prep, ph.mm, ph.resid = prep, mm, resid
            ph.kind = "fin"
            return ph

        phases = []
        for rep in range(reps):
            for blk in range(depth):
                phases.append(tok_phase(blk))
                phases.append(ch_phase(blk))
            phases.append(fin_phase())

        steps = []
        for pi, ph in enumerate(phases):
            if ph.kind == "fin" and pi > 0 and phases[pi - 1].kind == "ch":
                phases[pi - 1].zipf = ph
                continue
            for s in range(SPC):
                steps.append((ph, s))

        handles = {}

        def ensure_loaded(j):
            """Kick off a phase's weight DMAs well before its first step so
            the loads hide under the previous phase's (fast fp8) compute."""
            if j >= len(steps):
                return
            ph = steps[j][0]
            if not getattr(ph, "loaded", False):
                ph.load_small()
                ph.load_big()
                ph.loaded = True

        def emit_prep(j):
            if j >= len(steps):
                return
            ph, s = steps[j]
            handles[j] = ph.prep(s)

        # Warmup: sample-0's x DMAs must beat the bulk weight prefetch into
        # the (FIFO) DMA queue: x loads, phase-0 weights, then the lookahead
        # prefetch of later phases.
        ph0 = steps[0][0]
        ph0.load_small()
        emit_prep(0)
        emit_prep(1)
        ph0.load_big()
        ph0.loaded = True
        emit_prep(2)
        emit_prep(3)
        for jj in range(1, 9):
            ensure_loaded(jj)
        pend = None        # (resid_fn, s, accs) awaiting residual add
        for j, (ph, s) in enumerate(steps):
            ensure_loaded(j + 8)
            if j + 2 not in handles and j + 2 < len(steps):
                emit_prep(j + 2)
            zipf = getattr(ph, "zipf", None)
            fh = None
            if zipf is not None and s >= 0:
                if not getattr(zipf, "loaded", False):
                    zipf.load_small()
                    zipf.load_big()
                    zipf.loaded = True
                if s >= 2:
                    fh = zipf.prep(s - 2)
            if pend is not None:
                pend[0](pend[1], pend[2])
                pend = None
            accs = ph.mm(s, handles.pop(j))
            if accs is not None:
                pend = (ph.resid, s, accs)
            if fh is not None:
                zipf.mm(s - 2, fh)
            if zipf is not None and s == SPC - 1:
                if pend is not None:
                    pend[0](pend[1], pend[2])
                    pend = None
                fhs = [zipf.prep(s2) for s2 in (SPC - 2, SPC - 1)]
                for s2, fh2 in zip((SPC - 2, SPC - 1), fhs):
                    zipf.mm(s2, fh2)
        if pend is not None:
            pend[0](pend[1], pend[2])

    nc.compile()
    return nc


def _host_prep(inputs):
    g = {k: np.asarray(v, dtype=np.float32) for k, v in inputs.items()}
    ln1_triv = bool(np.all(g["ln1_g"] == 1.0) and np.all(g["ln1_b"] == 0.0))
    ln2_triv = bool(np.all(g["ln2_g"] == 1.0) and np.all(g["ln2_b"] == 0.0))
    lnf_triv = bool(np.all(g["lnf_g"] == 1.0) and np.all(g["lnf_b"] == 0.0))
    lb_triv = bool(np.all(g["lb"] == 0.0))
    b1_triv = bool(np.all(g["rb1"] == 0.0) and np.all(g["cb1"] == 0.0))
    b2_triv = bool(np.all(g["rb2"] == 0.0) and np.all(g["cb2"] == 0.0))
    flags = (ln1_triv, ln2_triv, lnf_triv, lb_triv, b1_triv, b2_triv)

    dmask = np.ones((NR, P, DIM), dtype=np.float32)
    for r in range(NR):
        for p in range(min(P, DIM - 128 * r)):
            dmask[r, p, 128 * r + p] = 0.0

    def bcast2(gv, bv):
        gb = np.stack([gv, bv], axis=-2)[..., None, :]
        return np.broadcast_to(gb, gb.shape[:-2] + (P, DIM)).copy()

    bf = ml_dtypes.bfloat16
    f8 = ml_dtypes.float8_e4m3

    def split_w1(w):  # [DEPTH, DIM, TOK] -> hi/lo [DEPTH, NR, P, TOK]
        wp = np.zeros((DEPTH, NR * P, TOK), np.float32)
        wp[:, :DIM] = w * WS
        hi = wp.astype(f8)
        lo = (wp - hi.astype(np.float32)).astype(f8)
        return (np.ascontiguousarray(hi.reshape(DEPTH, NR, P, TOK)),
                np.ascontiguousarray(lo.reshape(DEPTH, NR, P, TOK)))

    def split_w2(w):  # [DEPTH, TOK, DIM] -> hi/lo [DEPTH, NT, P, DIM]
        ws = w * WS
        hi = ws.astype(f8)
        lo = (ws - hi.astype(np.float32)).astype(f8)
        return (np.ascontiguousarray(hi.reshape(DEPTH, NT, P, DIM)),
                np.ascontiguousarray(lo.reshape(DEPTH, NT, P, DIM)))

    rw1h, rw1l = split_w1(g["rw1"])
    rw2h, rw2l = split_w2(g["rw2"])
    cw1h, cw1l = split_w1(g["cw1"])
    cw2h, cw2l = split_w2(g["cw2"])
    common = {
        "rw1h": rw1h, "rw1l": rw1l, "rw2h": rw2h, "rw2l": rw2l,
        "cw1h": cw1h, "cw1l": cw1l, "cw2h": cw2h, "cw2l": cw2l,
        "lw": np.ascontiguousarray(g["lw"].astype(bf)),
        "dmask": dmask,
    }
    if not b1_triv:
        common["rb1"] = g["rb1"]
        common["cb1"] = g["cb1"]
    if not b2_triv:
        common["rb2"] = g["rb2"]
        common["cb2b"] = np.ascontiguousarray(
            np.broadcast_to(g["cb2"][:, None, :], (DEPTH, P, DIM)))
    if not ln1_triv:
        common["ln1gb"] = np.ascontiguousarray(bcast2(g["ln1_g"], g["ln1_b"]))
    if not ln2_triv:
        common["ln2gb"] = np.ascontiguousarray(bcast2(g["ln2_g"], g["ln2_b"]))
    if not lnf_triv:
        common["lnfgb"] = np.ascontiguousarray(bcast2(g["lnf_g"], g["lnf_b"]))
    if not lb_triv:
        common["lbb"] = np.broadcast_to(g["lb"][None, :], (P, DIM)).copy()
    x = np.ascontiguousarray(g["x"])
    in_maps = [dict(common, x=np.ascontiguousarray(x[c * SPC:(c + 1) * SPC]))
               for c in range(NCORES)]
    return flags, in_maps


def _get_nc(flags, reps=1):
    key = flags + (reps,)
    if key not in _BUILD_CACHE:
        _BUILD_CACHE[key] = _build(*flags, reps=reps)
    return _BUILD_CACHE[key]


def kernel(**inputs):
    flags, in_maps = _host_prep(inputs)
    nc = _get_nc(flags)
    res = run_bass_kernel_spmd(nc, in_maps, list(range(NCORES)))
    return np.concatenate([res.results[c]["out"] for c in range(NCORES)], axis=0)


def measure_hw_time(inputs, r_hi=4, iters=5):
    """Wall-clock repetition-slope timing on the real device."""
    import time as _time
    flags, in_maps = _host_prep(inputs)
    cores = list(range(NCORES))
    t = {}
    for r in (1, r_hi):
        nc = _get_nc(flags, reps=r)
        best = float("inf")
        run_bass_kernel_spmd(nc, in_maps, cores)  # warm (jit + neff cache)
        for _ in range(iters):
            t0 = _time.perf_counter()
            run_bass_kernel_spmd(nc, in_maps, cores)
            best = min(best, _time.perf_counter() - t0)
        t[r] = best
        print(f"  reps={r}: best wall {best*1e3:.1f} ms")
    return (t[r_hi] - t[1]) / (r_hi - 1) * 1e9


def kernel_traced(**inputs):
    """Like kernel() but with NTFF tracing when available."""
    flags, in_maps = _host_prep(inputs)
    nc = _get_nc(flags)
    try:
        res = run_bass_kernel_spmd(nc, in_maps, list(range(NCORES)), trace=True)
    except ModuleNotFoundError:
        res = run_bass_kernel_spmd(nc, in_maps, list(range(NCORES)))
    out = np.concatenate([res.results[c]["out"] for c in range(NCORES)], axis=0)
    return out, res
